# revision 17
# baseline (speedup 1.0000x reference)
"""Trainium2 Bass kernel for PointConv-style e3nn message passing.

Self-contained: builds + runs an 8-core SPMD Bass kernel via
bass_utils.run_bass_kernel_spmd, accepting FULL inputs and returning the
FULL output.

Design (v3):
- Nodes padded to 20480, split 8 ways (2560/core); edges sorted by dst and
  assigned to the core owning the destination.
- Per core, destinations are processed in 5 groups of 512 nodes. Edges of a
  group are packed into T 128-edge tiles on a uniform column grid (each tile
  owns a baked 32-column window of the group's 512 psum columns), so the
  scatter-add becomes per-tile compact one-hot matmuls into 5 psum banks.
- The a0/a1 spherical-harmonic factors are folded into host-prescaled
  one-hots (oh*a0, oh*a1_d), so the device only forms the w*g products.
- v3 change vs v2: no replicated h-table and no device dma_gather. The host
  pre-gathers x[src] per edge slot (channel-major, slot order) and the
  device computes h = lin1(x) per 128-edge tile with two blockdiag matmuls
  directly into psum, alongside the radial tp-weight matmul.
"""

import os
import sys
import types
import ctypes

import numpy as np

import concourse.bass as bass
import concourse.bacc as bacc
import concourse.tile as tile
from concourse import mybir
from concourse.bass import AP
from concourse.bass_utils import run_bass_kernel_spmd

# ---------------------------------------------------------------- constants
N = 20000
E = 160000
MUL = 64
EDIM = 8
NZ = 4
AVG_NEIGH = 8.0
INV_SQRT3 = float(1.0 / np.sqrt(3.0))

CORES = 8
NP_PAD = 20480            # padded node count
NPC = NP_PAD // CORES     # 2560 nodes per core
GRP = 512                 # nodes per scatter group (psum bank width)
NG = NPC // GRP           # 5 groups per core
SPAN = 32                 # onehot column window per edge tile
BACK = 8                  # grid look-back
STW = 8                   # supertile width (tiles per DVE batch)

F32 = mybir.dt.float32
BF16 = mybir.dt.bfloat16
I32 = mybir.dt.int32
NP_BF16 = mybir.dt.np(mybir.dt.bfloat16)

LAST_RESULT = None


# ------------------------------------------------------- axon profile hook
def _install_profile_hook():
    """Make trace=True / BASS_TRACE=1 work under axon (degrades silently)."""
    if "antenv.axon_hooks" in sys.modules:
        return
    try:
        try:
            from trn_agent_boot.trn_boot import _ntff_profile_via_ctypes
        except ImportError:
            sys.path.insert(0, "/root/.axon_site")
            from trn_agent_boot.trn_boot import _ntff_profile_via_ctypes
        so_path = "/opt/axon/libaxon_pjrt.so"
        lib = ctypes.CDLL(so_path)
        if not hasattr(lib, "axon_start_nrt_profile"):
            return
        hook = _ntff_profile_via_ctypes(so_path)
        mod = types.ModuleType("antenv.axon_hooks")
        state = {"hook": hook}
        mod.set_axon_ntff_profile_hook = lambda h: state.__setitem__("hook", h)
        mod.get_axon_ntff_profile_hook = lambda: state["hook"]
        sys.modules["antenv.axon_hooks"] = mod
        import antenv
        antenv.axon_hooks = mod
    except Exception:
        pass


# ----------------------------------------------- tile-exit drain workaround
def _patch_tile_drain():
    """This toolchain's walrus rejects >1 sem wait on a Drain; hang the exit
    waits on a NoOp chain instead (bacc's generate_event_semaphores then
    legalises them)."""
    from concourse.vector_clock import ScopedClock

    def _drain_and_barrier(self, tick_clock, wait_clock):
        nop_inst = self.nc.sync.nop(nofuse=True, hint="tile_exit_wait")
        wait_clock.add_sem_waits(
            nop_inst.ins, ScopedClock({None: tick_clock.global_clock})
        )
        self.nc.sync.drain()
        self.nc.all_engine_barrier()
        assert self.sems is not None
        popped = self.nc._tile_sem_poison_stack.pop()
        assert popped is self._sem_poison
        self.nc.clear_and_free_semaphores(list(self.sems.allocated().values()))
        self.nc.all_engine_barrier()

    tile.TileContext._drain_and_barrier = _drain_and_barrier


_patch_tile_drain()


def _grid_starts(T):
    return [max(0, min(int(round(i * GRP / T)) - BACK, GRP - SPAN))
            for i in range(T)]


def _apv(base_ap, col_off, dims):
    """AP view of a 2D sbuf/psum tile: partitions x custom free dims."""
    pstep, pcount = base_ap.ap[0]
    return AP(base_ap.tensor, base_ap.offset + col_off,
              [[pstep, pcount]] + dims)


# ---------------------------------------------------------------- program
def _build_program(T):
    """Build the SPMD Bass program for T edge tiles per 512-node group."""
    C = T * 128               # edge slots per group
    skip = set(os.environ.get("BASS_SKIP", "").split(","))

    nc = bacc.Bacc(num_swdge_queues=4)

    # inputs (per core)
    xeT = nc.dram_tensor("xeT", [NG, 256, C], BF16, kind="ExternalInput")
    xTb = nc.dram_tensor("xTb", [256, NPC], BF16, kind="ExternalInput")
    ydram = nc.dram_tensor("ydram", [NG, 128, 8 * 512], BF16,
                           kind="ExternalInput")
    ohd = nc.dram_tensor("ohd", [NG, 128, T * 4 * SPAN], BF16,
                         kind="ExternalInput")
    NT3 = (T + 2) // 3
    CB = NT3 * 128
    embd = nc.dram_tensor("embd", [NG, 128, CB], BF16, kind="ExternalInput")
    wblk1_d = nc.dram_tensor("wblk1", [128, 128], BF16, kind="ExternalInput")
    wblk2_d = nc.dram_tensor("wblk2", [128, 128], BF16, kind="ExternalInput")
    wm1_d = nc.dram_tensor("wm1", [128, 128], BF16, kind="ExternalInput")
    wbig_d = nc.dram_tensor("wbig", [128, 256], BF16, kind="ExternalInput")
    w20a_d = nc.dram_tensor("w20a_p", [128, 128], BF16, kind="ExternalInput")
    w20b_d = nc.dram_tensor("w20b_p", [128, 128], BF16, kind="ExternalInput")
    w21at_d = nc.dram_tensor("w21a_t", [128, 64], BF16, kind="ExternalInput")
    w21bt_d = nc.dram_tensor("w21b_t", [128, 64], BF16, kind="ExternalInput")
    w21bb_d = nc.dram_tensor("w21b_b", [128, 64], BF16, kind="ExternalInput")
    wsc0_d = nc.dram_tensor("wsc0", [2, 128, 128], BF16, kind="ExternalInput")
    wsc1_d = nc.dram_tensor("wsc1", [2, 128, 64], BF16, kind="ExternalInput")
    outT = nc.dram_tensor("outT", [256, NPC], BF16, kind="ExternalOutput")

    ACT_SILU = mybir.ActivationFunctionType.Silu
    ACT_COPY = mybir.ActivationFunctionType.Copy
    MULT = mybir.AluOpType.mult

    starts = _grid_starts(T)

    with tile.TileContext(nc) as tc:
        with (
            tc.tile_pool(name="const", bufs=1) as cp,
            tc.tile_pool(name="grp", bufs=2) as gp,
            tc.tile_pool(name="hid5", bufs=5) as hp5,
            tc.tile_pool(name="xe", bufs=2) as xep,
            tc.tile_pool(name="st", bufs=3) as sp,
            tc.tile_pool(name="sts", bufs=2) as stsp,
            tc.tile_pool(name="node", bufs=2) as npl,
            tc.tile_pool(name="bank", bufs=1, space="PSUM") as bkp,
            tc.tile_pool(name="wps", bufs=3, space="PSUM") as wpp,
        ):
            # ---- constants
            wblk1 = cp.tile([128, 128], BF16)
            nc.sync.dma_start(out=wblk1[:], in_=wblk1_d[:])
            wblk2 = cp.tile([128, 128], BF16)
            nc.sync.dma_start(out=wblk2[:], in_=wblk2_d[:])
            wm1 = cp.tile([128, 128], BF16)
            nc.sync.dma_start(out=wm1[:], in_=wm1_d[:])
            wbig = cp.tile([128, 256], BF16)
            nc.sync.dma_start(out=wbig[:], in_=wbig_d[:])
            w20a_p = cp.tile([128, 128], BF16)
            nc.sync.dma_start(out=w20a_p[:], in_=w20a_d[:])
            w20b_p = cp.tile([128, 128], BF16)
            nc.sync.dma_start(out=w20b_p[:], in_=w20b_d[:])
            w21a_t = cp.tile([128, 64], BF16)
            nc.sync.dma_start(out=w21a_t[:], in_=w21at_d[:])
            w21b_t = cp.tile([128, 64], BF16)
            nc.sync.dma_start(out=w21b_t[:], in_=w21bt_d[:])
            w21b_b = cp.tile([128, 64], BF16)
            nc.sync.dma_start(out=w21b_b[:], in_=w21bb_d[:])
            wsc0a = cp.tile([128, 128], BF16)
            nc.sync.dma_start(out=wsc0a[:], in_=wsc0_d[0])
            wsc0b = cp.tile([128, 128], BF16)
            nc.sync.dma_start(out=wsc0b[:], in_=wsc0_d[1])
            wsc1a = cp.tile([128, 64], BF16)
            nc.sync.dma_start(out=wsc1a[:], in_=wsc1_d[0])
            wsc1b = cp.tile([128, 64], BF16)
            nc.sync.dma_start(out=wsc1b[:], in_=wsc1_d[1])
            zeros = cp.tile([128, 512], BF16)
            nc.vector.memset(zeros[:], 0.0)

            # ============ radial MLP for all groups (independent of x) ====
            # blockdiag3(Wm1) packing: 3 tiles share partitions at 32-part
            # spacing (p = 32*(t%3)+j; base partition must be 0/32/64).
            # hid_g[32*(t%3)+u, 128*(t//3)+n] is the tp-weight for slot
            # t*128 + n.
            hids = []
            for g in range(NG):
                embt = gp.tile([128, CB], BF16, tag="embt")
                nc.sync.dma_start(out=embt[:], in_=embd[g])
                hid = hp5.tile([128, CB], BF16, tag="hid", name=f"hid{g}")
                for c0 in range(0, CB, 512):
                    sz = min(512, CB - c0)
                    hp = wpp.tile([128, 512], F32, tag="wpb")
                    nc.tensor.matmul(hp[:, :sz], lhsT=wm1[:],
                                     rhs=embt[:, c0:c0 + sz],
                                     start=True, stop=True)
                    nc.scalar.activation(hid[:, c0:c0 + sz], hp[:, :sz],
                                         ACT_SILU)
                hids.append(hid)

            # scatter psum banks (reused across groups)
            def bank_tiles():
                return [bkp.tile([128, 512], F32, tag=f"bank{k}",
                                 name=f"bank{k}") for k in range(5)]

            # ================= per-group edge + node phases ================
            cpcnt = [0]
            for g in range(NG):
                banks = bank_tiles()
                if "memset" not in skip:
                    for k in range(5):
                        if k % 2 == 0:
                            nc.vector.memset(banks[k][:], 0.0)
                        else:
                            nc.scalar.activation(banks[k][:], zeros[:],
                                                 ACT_COPY)

                hid = hids[g]

                # per-slot x inputs (channel-major, slot order)
                xe0 = xep.tile([128, C], BF16, tag="xe0", name=f"xe0_{g}")
                nc.sync.dma_start(out=xe0[:], in_=xeT[g][0:128, :])
                xe1 = xep.tile([128, C], BF16, tag="xe1", name=f"xe1_{g}")
                nc.sync.dma_start(out=xe1[:], in_=xeT[g][128:256, :])

                # prefetch node-phase inputs for this group
                cols = slice(g * GRP, (g + 1) * GRP)
                xga = npl.tile([128, 512], BF16, tag="xga")
                nc.sync.dma_start(out=xga[:], in_=xTb[0:128, cols])
                xgb = npl.tile([128, 512], BF16, tag="xgb")
                nc.sync.dma_start(out=xgb[:], in_=xTb[128:256, cols])
                ytile = npl.tile([128, 8 * 512], BF16, tag="ytile")
                nc.sync.dma_start(out=ytile[:], in_=ydram[g])

                # edge supertiles
                pend = None          # deferred scatter work (prev supertile)

                def emit_scatter(work):
                    st0, L0, wgt0, oht0 = work
                    for lt in ([] if "scatter" in skip else range(L0)):
                        t = st0 + lt
                        col0 = starts[t]
                        wb = lt * 640
                        ob = lt * 4 * SPAN
                        last = (t == T - 1)
                        # bankP: [s0a|s1b_0] <- [A|B0] x oh_a0
                        nc.tensor.matmul(
                            banks[0][:, col0:col0 + SPAN],
                            lhsT=wgt0[:, wb:wb + 128],
                            rhs=oht0[:, ob:ob + SPAN],
                            start=False, stop=last, skip_group_check=True)
                        # bankQ: [s1b_1|s1b_2] <- [B1|B2] x oh_a0
                        nc.tensor.matmul(
                            banks[1][:, col0:col0 + SPAN],
                            lhsT=wgt0[:, wb + 128:wb + 256],
                            rhs=oht0[:, ob:ob + SPAN],
                            start=False, stop=last, skip_group_check=True)
                        # bankR_d: [s1a_d|s0b_d] <- [C|D_d] x oh_a1_d
                        for d in range(3):
                            nc.tensor.matmul(
                                banks[2 + d][:, col0:col0 + SPAN],
                                lhsT=wgt0[:, wb + 256 + 128 * d:
                                         wb + 384 + 128 * d],
                                rhs=oht0[:, ob + SPAN * (1 + d):
                                        ob + SPAN * (2 + d)],
                                start=False, stop=last,
                                skip_group_check=True)

                for st in range(0, T, STW):
                    L = min(STW, T - st)
                    oht = sp.tile([128, STW * 4 * SPAN], BF16, tag="oht")
                    nc.sync.dma_start(
                        out=oht[:, :L * 4 * SPAN],
                        in_=ohd[g][:, st * 4 * SPAN:(st + L) * 4 * SPAN])

                    # per-tile [h(256) | wp(256)] interleaved, bf16
                    hw_sb = sp.tile([128, STW * 512], BF16, tag="hw_sb")

                    # per-tile: lin1 + wp matmuls into one psum bank, then
                    # psum->sbuf copies split across scalar/gpsimd
                    for lt in range(L):
                        t = st + lt
                        hwp = wpp.tile([128, 512], F32, tag="wpb",
                                       name=f"hwp_{g}_{t}")
                        if "lin1" not in skip:
                            nc.tensor.matmul(hwp[:, 0:128],
                                             lhsT=xe0[:, 128 * t:128 * t + 128],
                                             rhs=wblk1[:],
                                             start=True, stop=True)
                            nc.tensor.matmul(hwp[:, 128:256],
                                             lhsT=xe1[:, 128 * t:128 * t + 128],
                                             rhs=wblk2[:],
                                             start=True, stop=True)
                        if "wp" not in skip:
                            hb = 32 * (t % 3)
                            nc.tensor.matmul(hwp[:, 256:512],
                                             lhsT=hid[hb:hb + 8,
                                                      128 * (t // 3):
                                                      128 * (t // 3) + 128],
                                             rhs=wbig[hb:hb + 8, :],
                                             start=True, stop=True)
                        # psum->sbuf copies (only scalar/DVE may read PSUM);
                        # one [128,512] op per bank, alternating engines
                        dst = hw_sb[:, 512 * lt:512 * lt + 512]
                        if cpcnt[0] % 5 in (0, 2, 4):
                            nc.scalar.activation(dst, hwp[:], ACT_COPY)
                        else:
                            nc.vector.tensor_copy(out=dst, in_=hwp[:])
                        cpcnt[0] += 1

                    # wg products: [A|B0|B1|B2|C|D0|C|D1|C|D2] per tile
                    # layout: 640 cols/tile: A(64) B_d(192) [C|D_d](3x128)
                    wgt = sp.tile([128, STW * 640], BF16, tag="wgt", bufs=2)
                    if "wg" in skip:
                        pass
                    else:
                      # A = w1*g0 (gpsimd, sbuf only)
                      nc.gpsimd.tensor_tensor(
                        out=_apv(wgt[:], 0, [[640, L], [1, 64]]),
                        in0=_apv(hw_sb[:], 256, [[512, L], [1, 64]]),
                        in1=_apv(hw_sb[:], 0, [[512, L], [1, 64]]),
                        op=MULT)
                      # B_d = w3*g1_d
                      nc.vector.tensor_tensor(
                        out=_apv(wgt[:], 64, [[640, L], [64, 3], [1, 64]]),
                        in0=_apv(hw_sb[:], 256 + 64,
                                 [[512, L], [0, 3], [1, 64]]),
                        in1=_apv(hw_sb[:], 64, [[512, L], [64, 3], [1, 64]]),
                        op=MULT)
                      # C = w2*g0 (replicated 3x at 256+128d)
                      ceng = nc.gpsimd if (st // STW) % 2 == 1 else nc.vector
                      ceng.tensor_tensor(
                        out=_apv(wgt[:], 256, [[640, L], [128, 3], [1, 64]]),
                        in0=_apv(hw_sb[:], 256 + 128,
                                 [[512, L], [0, 3], [1, 64]]),
                        in1=_apv(hw_sb[:], 0, [[512, L], [0, 3], [1, 64]]),
                        op=MULT)
                      # D_d = w4'*g1_d (at 320+128d)
                      nc.vector.tensor_tensor(
                        out=_apv(wgt[:], 320, [[640, L], [128, 3], [1, 64]]),
                        in0=_apv(hw_sb[:], 256 + 192,
                                 [[512, L], [0, 3], [1, 64]]),
                        in1=_apv(hw_sb[:], 64, [[512, L], [64, 3], [1, 64]]),
                        op=MULT)

                    # compact scatter matmuls (deferred one supertile so the
                    # PE can pipeline lin1/wp of st+1 with wg of st)
                    if pend is not None:
                        emit_scatter(pend)
                    pend = (st, L, wgt, oht)
                if pend is not None:
                    emit_scatter(pend)

                # drain banks -> sts (bf16)
                sts = []
                for k in range(5):
                    stile = stsp.tile([128, 512], BF16, tag=f"sts{k}",
                                      name=f"sts{k}_{g}")
                    if k % 2 == 0:
                        nc.scalar.activation(stile[:], banks[k][:], ACT_COPY)
                    else:
                        nc.vector.tensor_copy(out=stile[:], in_=banks[k][:])
                    sts.append(stile)
                stsP, stsQ, stsR = sts[0], sts[1], sts[2:5]

                # ---------------- node phase for this group ----------------
                if "node" in skip:
                    outa = npl.tile([128, 512], BF16, tag="outa")
                    nc.vector.tensor_copy(out=outa[:], in_=xga[:])
                    outb = npl.tile([128, 512], BF16, tag="outb")
                    nc.vector.tensor_copy(out=outb[:], in_=xgb[:])
                else:
                    # up0 = W20^T s0 + sc0   [scalars|gates, 512]
                    up0 = bkp.tile([128, 512], F32, tag="bank0",
                                   name=f"up0_{g}")
                    sRt = npl.tile([128, 512], BF16, tag="sRt")
                    nc.vector.tensor_add(out=sRt[:], in0=stsR[0][:],
                                         in1=stsR[1][:])
                    nc.vector.tensor_add(out=sRt[:], in0=sRt[:],
                                         in1=stsR[2][:])
                    nc.tensor.matmul(up0[:], lhsT=w20a_p[:],
                                     rhs=stsP[:], start=True, stop=False)
                    nc.tensor.matmul(up0[:], lhsT=w20b_p[:],
                                     rhs=sRt[:],
                                     start=False, stop=False)
                    nc.tensor.matmul(up0[:], lhsT=wsc0a[:],
                                     rhs=ytile[:, 0:512],
                                     start=False, stop=False)
                    nc.tensor.matmul(up0[:], lhsT=wsc0b[:],
                                     rhs=ytile[:, 512:1024],
                                     start=False, stop=True)

                    # up1: d0 rows 0:64, d1 rows 64:128 of up1a; d2 in up1b
                    up1a = bkp.tile([128, 512], F32, tag="bank1",
                                     name=f"up1a_{g}")
                    for d in (0, 1):
                        rows = slice(64 * d, 64 * d + 64)
                        if d == 0:
                            s1b_src, s1b_w = stsP[:], w21b_b[:]
                        else:
                            s1b_src, s1b_w = stsQ[:], w21b_t[:]
                        nc.tensor.matmul(up1a[rows, :], lhsT=w21a_t[:],
                                         rhs=stsR[d][:],
                                         start=True, stop=False)
                        nc.tensor.matmul(up1a[rows, :], lhsT=s1b_w, rhs=s1b_src,
                                         start=False, stop=False)
                        yo = 1024 * (1 + d)
                        nc.tensor.matmul(up1a[rows, :], lhsT=wsc1a[:],
                                         rhs=ytile[:, yo:yo + 512],
                                         start=False, stop=False)
                        nc.tensor.matmul(up1a[rows, :], lhsT=wsc1b[:],
                                         rhs=ytile[:, yo + 512:yo + 1024],
                                         start=False, stop=True)

                    # gate scalars/gates -> bf16
                    t0s = npl.tile([128, 512], BF16, tag="t0s")
                    nc.scalar.activation(t0s[:], up0[:], ACT_SILU)

                    up1b = wpp.tile([64, 512], F32, tag="wpb",
                                    name=f"up1b_{g}")
                    nc.tensor.matmul(up1b[:], lhsT=w21a_t[:],
                                     rhs=stsR[2][:], start=True, stop=False)
                    nc.tensor.matmul(up1b[:], lhsT=w21b_b[:],
                                     rhs=stsQ[:], start=False, stop=False)
                    nc.tensor.matmul(up1b[:], lhsT=wsc1a[:],
                                     rhs=ytile[:, 3072:3584],
                                     start=False, stop=False)
                    nc.tensor.matmul(up1b[:], lhsT=wsc1b[:],
                                     rhs=ytile[:, 3584:4096],
                                     start=False, stop=True)

                    # assemble: vectors = gates*t1 (t1 from psum); resnet add
                    outa = npl.tile([128, 512], BF16, tag="outa")
                    nc.vector.tensor_add(out=outa[0:64, :], in0=t0s[0:64, :],
                                         in1=xga[0:64, :])
                    nc.vector.tensor_tensor(out=outa[64:128, :],
                                            in0=t0s[64:128, :],
                                            in1=up1a[0:64, :], op=MULT)
                    nc.vector.tensor_add(out=outa[64:128, :], in0=outa[64:128, :],
                                         in1=xga[64:128, :])
                    outb = npl.tile([128, 512], BF16, tag="outb")
                    nc.vector.tensor_tensor(out=outb[0:64, :],
                                            in0=t0s[64:128, :],
                                            in1=up1a[64:128, :], op=MULT)
                    nc.vector.tensor_add(out=outb[0:64, :], in0=outb[0:64, :],
                                         in1=xgb[0:64, :])
                    nc.vector.tensor_tensor(out=outb[64:128, :],
                                            in0=t0s[64:128, :],
                                            in1=up1b[:], op=MULT)
                    nc.vector.tensor_add(out=outb[64:128, :], in0=outb[64:128, :],
                                         in1=xgb[64:128, :])

                nc.sync.dma_start(out=outT[0:128, cols], in_=outa[:])
                nc.sync.dma_start(out=outT[128:256, cols], in_=outb[:])

    nc.compile()
    return nc


# ---------------------------------------------------------------- host prep
def _pack_group(cols, T):
    """Greedy pack of sorted dst-cols into T tiles on the uniform grid.
    Returns per-tile edge index lists (positions into cols) or None."""
    starts = _grid_starts(T)
    res = []
    j, nE = 0, len(cols)
    for t in range(T):
        lo, hi = starts[t], starts[t] + SPAN
        tl = []
        while j < nE and len(tl) < 128 and cols[j] < hi:
            if cols[j] < lo:
                return None
            tl.append(j)
            j += 1
        res.append(tl)
    if j < nE:
        return None
    return res


def _host_prep(node_feats, node_attrs, edge_attrs, edge_embedding,
               W_lin1_0, W_lin1_1, W_mlp1, W_mlp2,
               W_lin2_0, W_lin2_1, W_sc0, W_sc1, edge_index):
    inv = 1.0 / np.sqrt(MUL)
    inv_e = 1.0 / np.sqrt(EDIM)
    inv2 = 1.0 / np.sqrt(2 * MUL)
    inv_n = 1.0 / np.sqrt(AVG_NEIGH)
    inv_sc = 1.0 / np.sqrt(MUL * NZ)

    # channel permutation: ours = [x0(64) | x1 d-major(192)]
    gidx = np.empty(256, np.int64)
    gidx[:64] = np.arange(64)
    for d in range(3):
        for u in range(64):
            gidx[64 + 64 * d + u] = 64 + 3 * u + d

    # permuted node feats in bf16 (row N = zero pad row for empty slots)
    xg_pad = np.zeros((N + 1, 256), np.float32)
    xg_pad[:N] = node_feats[:, gidx]
    xg_pad_b = xg_pad.astype(NP_BF16)

    xgf = np.zeros((NP_PAD, 256), np.float32)
    xgf[:N] = node_feats[:, gidx]
    xT = np.ascontiguousarray(xgf.T)
    xTf = xT.astype(NP_BF16)

    attT = np.zeros((NZ, NP_PAD), np.float32)
    attT[:, :N] = node_attrs.T.astype(np.float32)

    # ---- edge sorting and per-(core,group) packing
    src = edge_index[0].astype(np.int64)
    dst = edge_index[1].astype(np.int64)
    order = np.argsort(dst, kind="stable")
    src_s, dst_s = src[order], dst[order]
    ea_s = edge_attrs[order].astype(np.float32)
    emb_s = edge_embedding[order].astype(np.float32)

    bounds = np.searchsorted(dst_s, np.arange(0, NP_PAD + 1, GRP))
    all_cols = []
    T = 2
    for c in range(CORES):
        for g in range(NG):
            gi = c * NG + g
            s, e = bounds[gi], bounds[gi + 1]
            cols = (dst_s[s:e] - gi * GRP).astype(int)
            all_cols.append(cols)
            Tg = max(1, int(np.ceil(len(cols) / 128)))
            while Tg < 96 and _pack_group(cols, Tg) is None:
                Tg += 1
            T = max(T, Tg)
    T = T + (T % 2)  # even
    while any(_pack_group(cols, T) is None for cols in all_cols):
        T += 2

    C = T * 128

    per_core = []
    for c in range(CORES):
        slot_src = np.full((NG, C), N, np.int64)   # default -> zero row
        oh = np.zeros((NG, T, 128, 4 * SPAN), np.float32)
        embw = np.zeros((NG, EDIM, C), np.float32)
        starts = _grid_starts(T)
        for g in range(NG):
            gi = c * NG + g
            s = bounds[gi]
            cols = all_cols[gi]
            pk = _pack_group(cols, T)
            assert pk is not None
            for t, tl in enumerate(pk):
                if not tl:
                    continue
                idx = np.asarray(tl, np.int64)
                p = np.arange(len(tl))
                slot = t * 128 + p
                sn = src_s[s + idx]
                slot_src[g, slot] = sn
                embw[g, :, slot] = emb_s[s + idx]
                cc = cols[idx] - starts[t]
                oh[g, t, p, cc] = ea_s[s + idx, 0]               # oh*a0
                for d in range(3):
                    oh[g, t, p, SPAN * (1 + d) + cc] = ea_s[s + idx, 1 + d]
        # pre-gathered x per slot, channel-major [NG, 256, C]
        xe = xg_pad_b[slot_src.reshape(-1)]          # [NG*C, 256] bf16
        xe_dev = np.ascontiguousarray(
            xe.reshape(NG, C, 256).transpose(0, 2, 1))
        # device layout [NG, 128, T*4*SPAN]
        oh_dev = oh.transpose(0, 2, 1, 3).reshape(NG, 128, T * 4 * SPAN)
        # embd blockdiag3 layout: [NG, 32*(t%3)+j, 128*(t//3)+n] for
        # slot t*128 + n (rows 8:32 of each 32-block and 96:128 zero)
        NT3 = (T + 2) // 3
        embp = np.zeros((NG, EDIM, NT3 * 384), np.float32)
        embp[:, :, :C] = embw
        embp = embp.reshape(NG, EDIM, NT3, 3, 128)
        emb2 = np.zeros((NG, 4, 32, NT3, 128), np.float32)
        emb2[:, :3, :EDIM] = embp.transpose(0, 3, 1, 2, 4)
        emb2 = emb2.reshape(NG, 128, NT3 * 128)
        per_core.append(dict(
            xeT=xe_dev,
            ohd=np.ascontiguousarray(oh_dev).astype(NP_BF16),
            embd=np.ascontiguousarray(emb2).astype(NP_BF16),
        ))

    # ---- weights
    W10s = (W_lin1_0 * inv).astype(np.float32)
    W11s = (W_lin1_1 * inv).astype(np.float32)
    wblk1 = np.zeros((128, 128), np.float32)
    wblk1[:64, :64] = W10s
    wblk1[64:, 64:] = W11s
    wblk2 = np.zeros((128, 128), np.float32)
    wblk2[:64, :64] = W11s
    wblk2[64:, 64:] = W11s
    wm1s = (W_mlp1 * inv_e).astype(np.float32)
    wm1 = np.zeros((128, 128), np.float32)
    for q in range(3):
        wm1[32 * q:32 * q + EDIM, 32 * q:32 * q + EDIM] = wm1s
    wm1 = wm1.astype(NP_BF16)
    w1 = W_mlp2[:, 0:64]
    w2 = W_mlp2[:, 64:128]
    w3 = W_mlp2[:, 128:192]
    w4 = W_mlp2[:, 192:256]
    # wp cols: [w1 | w3 | w2 | w4'], replicated in 4 partition quadrants
    wbig8 = (np.concatenate([w1, w3, w2, w4 * INV_SQRT3], axis=1)
             * inv_e).astype(np.float32)
    wbig = np.zeros((128, 256), np.float32)
    for q in range(3):
        wbig[32 * q:32 * q + EDIM] = wbig8
    wbig = wbig.astype(NP_BF16)
    w20s = (W_lin2_0 * inv2 * inv_n).astype(np.float32)
    w21s = (W_lin2_1 * inv2 * inv_n).astype(np.float32)
    z64x128 = np.zeros((64, 128), np.float32)
    z64x64 = np.zeros((64, 64), np.float32)
    w20a_p = np.concatenate([w20s[0:64], z64x128]).astype(NP_BF16)
    w20b_p = np.concatenate([z64x128, w20s[64:128]]).astype(NP_BF16)
    w21a_t = np.concatenate([w21s[0:64], z64x64]).astype(NP_BF16)
    w21b_t = np.concatenate([w21s[64:128], z64x64]).astype(NP_BF16)
    w21b_b = np.concatenate([z64x64, w21s[64:128]]).astype(NP_BF16)
    wsc0z = (np.transpose(W_sc0, (1, 0, 2)).reshape(NZ * MUL, 2 * MUL)
             * inv_sc).astype(NP_BF16)
    wsc1z = (np.transpose(W_sc1, (1, 0, 2)).reshape(NZ * MUL, MUL)
             * inv_sc).astype(NP_BF16)
    wsc0 = np.stack([wsc0z[:128], wsc0z[128:]])
    wsc1 = np.stack([wsc1z[:128], wsc1z[128:]])

    shared = dict(wblk1=wblk1.astype(NP_BF16),
                  wblk2=wblk2.astype(NP_BF16), wm1=wm1, wbig=wbig,
                  w20a_p=w20a_p, w20b_p=w20b_p, w21a_t=w21a_t,
                  w21b_t=w21b_t, w21b_b=w21b_b, wsc0=wsc0, wsc1=wsc1)
    in_maps = []
    for c in range(CORES):
        m = dict(shared)
        csl = slice(c * NPC, (c + 1) * NPC)
        m["xTb"] = np.ascontiguousarray(xTf[:, csl])
        # host-precomputed self-connection inputs y = x_block * z_attr
        # block (k, h): cols 512*(2k+h), rows z*64+u (z = 2h + p//64)
        ycore = np.empty((128, NPC, 8), np.float32)
        xTc = xT[:, csl]
        atc = attT[:, csl]
        for k in range(4):
            xk = xTc[64 * k:64 * k + 64]
            for h in range(2):
                ycore[0:64, :, 2 * k + h] = xk * atc[2 * h]
                ycore[64:128, :, 2 * k + h] = xk * atc[2 * h + 1]
        yd = ycore.reshape(128, NG, 512, 8).transpose(1, 0, 3, 2).reshape(
            NG, 128, 8 * 512)
        m["ydram"] = np.ascontiguousarray(yd).astype(NP_BF16)
        m.update(per_core[c])
        in_maps.append(m)
    return T, in_maps, gidx


_PROGRAM_CACHE = {}


def kernel(**inputs):
    global LAST_RESULT
    _install_profile_hook()

    args = {k: np.asarray(v) for k, v in inputs.items()}
    T, in_maps, gidx = _host_prep(
        args["node_feats"].astype(np.float32),
        args["node_attrs"].astype(np.float32),
        args["edge_attrs"].astype(np.float32),
        args["edge_embedding"].astype(np.float32),
        args["W_lin1_0"].astype(np.float32),
        args["W_lin1_1"].astype(np.float32),
        args["W_mlp1"].astype(np.float32),
        args["W_mlp2"].astype(np.float32),
        args["W_lin2_0"].astype(np.float32),
        args["W_lin2_1"].astype(np.float32),
        args["W_sc0"].astype(np.float32),
        args["W_sc1"].astype(np.float32),
        args["edge_index"])

    if T not in _PROGRAM_CACHE:
        _PROGRAM_CACHE[T] = _build_program(T)
    nc = _PROGRAM_CACHE[T]

    trace = bool(int(os.environ.get("BASS_TRACE", "0")))
    res = run_bass_kernel_spmd(nc, in_maps, core_ids=list(range(CORES)),
                               trace=trace)
    LAST_RESULT = res

    outT = np.concatenate(
        [res.results[c]["outT"].astype(np.float32) for c in range(CORES)],
        axis=1)                            # [256, NP_PAD]
    full = outT.T[:N]                      # [N, 256] in our channel order
    out = np.empty((N, 256), np.float32)
    out[:, gidx] = full
    return out


# revision 18
# speedup vs baseline: 1.1253x; 1.1253x over previous
"""Trainium2 Bass kernel for PointConv-style e3nn message passing.

Self-contained: builds + runs an 8-core SPMD Bass kernel via
bass_utils.run_bass_kernel_spmd, accepting FULL inputs and returning the
FULL output.

Design (v3):
- Nodes padded to 20480, split 8 ways (2560/core); edges sorted by dst and
  assigned to the core owning the destination.
- Per core, destinations are processed in 5 groups of 512 nodes. Edges of a
  group are packed into T 128-edge tiles on a uniform column grid (each tile
  owns a baked 32-column window of the group's 512 psum columns), so the
  scatter-add becomes per-tile compact one-hot matmuls into 5 psum banks.
- The a0/a1 spherical-harmonic factors are folded into host-prescaled
  one-hots (oh*a0, oh*a1_d), so the device only forms the w*g products.
- v3 change vs v2: no replicated h-table and no device dma_gather. The host
  pre-gathers x[src] per edge slot (channel-major, slot order) and the
  device computes h = lin1(x) per 128-edge tile with two blockdiag matmuls
  directly into psum, alongside the radial tp-weight matmul.
"""

import os
import sys
import types
import ctypes

import numpy as np

import concourse.bass as bass
import concourse.bacc as bacc
import concourse.tile as tile
from concourse import mybir
from concourse.bass import AP
from concourse.bass_utils import run_bass_kernel_spmd

# ---------------------------------------------------------------- constants
N = 20000
E = 160000
MUL = 64
EDIM = 8
NZ = 4
AVG_NEIGH = 8.0
INV_SQRT3 = float(1.0 / np.sqrt(3.0))

CORES = 8
NP_PAD = 20480            # padded node count
NPC = NP_PAD // CORES     # 2560 nodes per core
GRP = 512                 # nodes per scatter group (psum bank width)
NG = NPC // GRP           # 5 groups per core
SPAN = 32                 # onehot column window per edge tile
BACK = 8                  # grid look-back
STW = 8                   # supertile width (tiles per DVE batch)

F32 = mybir.dt.float32
BF16 = mybir.dt.bfloat16
I32 = mybir.dt.int32
NP_BF16 = mybir.dt.np(mybir.dt.bfloat16)

LAST_RESULT = None


# ------------------------------------------------------- axon profile hook
def _install_profile_hook():
    """Make trace=True / BASS_TRACE=1 work under axon (degrades silently)."""
    if "antenv.axon_hooks" in sys.modules:
        return
    try:
        try:
            from trn_agent_boot.trn_boot import _ntff_profile_via_ctypes
        except ImportError:
            sys.path.insert(0, "/root/.axon_site")
            from trn_agent_boot.trn_boot import _ntff_profile_via_ctypes
        so_path = "/opt/axon/libaxon_pjrt.so"
        lib = ctypes.CDLL(so_path)
        if not hasattr(lib, "axon_start_nrt_profile"):
            return
        hook = _ntff_profile_via_ctypes(so_path)
        mod = types.ModuleType("antenv.axon_hooks")
        state = {"hook": hook}
        mod.set_axon_ntff_profile_hook = lambda h: state.__setitem__("hook", h)
        mod.get_axon_ntff_profile_hook = lambda: state["hook"]
        sys.modules["antenv.axon_hooks"] = mod
        import antenv
        antenv.axon_hooks = mod
    except Exception:
        pass


# ----------------------------------------------- tile-exit drain workaround
def _patch_tile_drain():
    """This toolchain's walrus rejects >1 sem wait on a Drain; hang the exit
    waits on a NoOp chain instead (bacc's generate_event_semaphores then
    legalises them)."""
    from concourse.vector_clock import ScopedClock

    def _drain_and_barrier(self, tick_clock, wait_clock):
        nop_inst = self.nc.sync.nop(nofuse=True, hint="tile_exit_wait")
        wait_clock.add_sem_waits(
            nop_inst.ins, ScopedClock({None: tick_clock.global_clock})
        )
        self.nc.sync.drain()
        self.nc.all_engine_barrier()
        assert self.sems is not None
        popped = self.nc._tile_sem_poison_stack.pop()
        assert popped is self._sem_poison
        self.nc.clear_and_free_semaphores(list(self.sems.allocated().values()))
        self.nc.all_engine_barrier()

    tile.TileContext._drain_and_barrier = _drain_and_barrier


_patch_tile_drain()


def _grid_starts(T):
    return [max(0, min(int(round(i * GRP / T)) - BACK, GRP - SPAN))
            for i in range(T)]


def _apv(base_ap, col_off, dims):
    """AP view of a 2D sbuf/psum tile: partitions x custom free dims."""
    pstep, pcount = base_ap.ap[0]
    return AP(base_ap.tensor, base_ap.offset + col_off,
              [[pstep, pcount]] + dims)


# ---------------------------------------------------------------- program
def _build_program(T):
    """Build the SPMD Bass program for T edge tiles per 512-node group."""
    C = T * 128               # edge slots per group
    skip = set(os.environ.get("BASS_SKIP", "").split(","))

    nc = bacc.Bacc(num_swdge_queues=4)

    # inputs (per core)
    xeT = nc.dram_tensor("xeT", [NG, 256, C], BF16, kind="ExternalInput")
    xTb = nc.dram_tensor("xTb", [256, NPC], BF16, kind="ExternalInput")
    ydram = nc.dram_tensor("ydram", [NG, 128, 8 * 512], BF16,
                           kind="ExternalInput")
    ohd = nc.dram_tensor("ohd", [NG, 128, T * 4 * SPAN], BF16,
                         kind="ExternalInput")
    NT3 = (T + 2) // 3
    CB = NT3 * 128
    embd = nc.dram_tensor("embd", [NG, 128, CB], BF16, kind="ExternalInput")
    wblk1_d = nc.dram_tensor("wblk1", [128, 128], BF16, kind="ExternalInput")
    wblk2_d = nc.dram_tensor("wblk2", [128, 128], BF16, kind="ExternalInput")
    wm1_d = nc.dram_tensor("wm1", [128, 128], BF16, kind="ExternalInput")
    wbig_d = nc.dram_tensor("wbig", [128, 256], BF16, kind="ExternalInput")
    w20a_d = nc.dram_tensor("w20a_p", [128, 128], BF16, kind="ExternalInput")
    w20b_d = nc.dram_tensor("w20b_p", [128, 128], BF16, kind="ExternalInput")
    w21at_d = nc.dram_tensor("w21a_t", [128, 64], BF16, kind="ExternalInput")
    w21bt_d = nc.dram_tensor("w21b_t", [128, 64], BF16, kind="ExternalInput")
    w21bb_d = nc.dram_tensor("w21b_b", [128, 64], BF16, kind="ExternalInput")
    wsc0_d = nc.dram_tensor("wsc0", [2, 128, 128], BF16, kind="ExternalInput")
    wsc1_d = nc.dram_tensor("wsc1", [2, 128, 64], BF16, kind="ExternalInput")
    outT = nc.dram_tensor("outT", [256, NPC], BF16, kind="ExternalOutput")

    ACT_SILU = mybir.ActivationFunctionType.Silu
    ACT_COPY = mybir.ActivationFunctionType.Copy
    MULT = mybir.AluOpType.mult

    starts = _grid_starts(T)

    with tile.TileContext(nc) as tc:
        with (
            tc.tile_pool(name="const", bufs=1) as cp,
            tc.tile_pool(name="grp", bufs=2) as gp,
            tc.tile_pool(name="hid5", bufs=5) as hp5,
            tc.tile_pool(name="xe", bufs=2) as xep,
            tc.tile_pool(name="st", bufs=3) as sp,
            tc.tile_pool(name="sts", bufs=2) as stsp,
            tc.tile_pool(name="node", bufs=2) as npl,
            tc.tile_pool(name="bank", bufs=1, space="PSUM") as bkp,
            tc.tile_pool(name="wps", bufs=3, space="PSUM") as wpp,
        ):
            # ---- constants
            wblk1 = cp.tile([128, 128], BF16)
            nc.sync.dma_start(out=wblk1[:], in_=wblk1_d[:])
            wblk2 = cp.tile([128, 128], BF16)
            nc.sync.dma_start(out=wblk2[:], in_=wblk2_d[:])
            wm1 = cp.tile([128, 128], BF16)
            nc.sync.dma_start(out=wm1[:], in_=wm1_d[:])
            wbig = cp.tile([128, 256], BF16)
            nc.sync.dma_start(out=wbig[:], in_=wbig_d[:])
            w20a_p = cp.tile([128, 128], BF16)
            nc.sync.dma_start(out=w20a_p[:], in_=w20a_d[:])
            w20b_p = cp.tile([128, 128], BF16)
            nc.sync.dma_start(out=w20b_p[:], in_=w20b_d[:])
            w21a_t = cp.tile([128, 64], BF16)
            nc.sync.dma_start(out=w21a_t[:], in_=w21at_d[:])
            w21b_t = cp.tile([128, 64], BF16)
            nc.sync.dma_start(out=w21b_t[:], in_=w21bt_d[:])
            w21b_b = cp.tile([128, 64], BF16)
            nc.sync.dma_start(out=w21b_b[:], in_=w21bb_d[:])
            wsc0a = cp.tile([128, 128], BF16)
            nc.sync.dma_start(out=wsc0a[:], in_=wsc0_d[0])
            wsc0b = cp.tile([128, 128], BF16)
            nc.sync.dma_start(out=wsc0b[:], in_=wsc0_d[1])
            wsc1a = cp.tile([128, 64], BF16)
            nc.sync.dma_start(out=wsc1a[:], in_=wsc1_d[0])
            wsc1b = cp.tile([128, 64], BF16)
            nc.sync.dma_start(out=wsc1b[:], in_=wsc1_d[1])
            zeros = cp.tile([128, 512], BF16)
            nc.vector.memset(zeros[:], 0.0)

            # ============ radial MLP for all groups (independent of x) ====
            # blockdiag3(Wm1) packing: 3 tiles share partitions at 32-part
            # spacing (p = 32*(t%3)+j; base partition must be 0/32/64).
            # hid_g[32*(t%3)+u, 128*(t//3)+n] is the tp-weight for slot
            # t*128 + n.
            hids = []
            for g in range(NG):
                embt = gp.tile([128, CB], BF16, tag="embt")
                nc.sync.dma_start(out=embt[:], in_=embd[g])
                hid = hp5.tile([128, CB], BF16, tag="hid", name=f"hid{g}")
                for c0 in range(0, CB, 512):
                    sz = min(512, CB - c0)
                    hp = wpp.tile([128, 512], F32, tag="wpb")
                    nc.tensor.matmul(hp[:, :sz], lhsT=wm1[:],
                                     rhs=embt[:, c0:c0 + sz],
                                     start=True, stop=True)
                    nc.scalar.activation(hid[:, c0:c0 + sz], hp[:, :sz],
                                         ACT_SILU)
                hids.append(hid)

            # scatter psum banks (reused across groups)
            def bank_tiles():
                return [bkp.tile([128, 512], F32, tag=f"bank{k}",
                                 name=f"bank{k}") for k in range(5)]

            # ================= per-group edge + node phases ================
            cpcnt = [0]
            for g in range(NG):
                banks = bank_tiles()
                if "memset" not in skip:
                    for k in range(5):
                        if k % 2 == 0:
                            nc.vector.memset(banks[k][:], 0.0)
                        else:
                            nc.scalar.activation(banks[k][:], zeros[:],
                                                 ACT_COPY)

                hid = hids[g]

                # per-slot x inputs (channel-major, slot order)
                xe0 = xep.tile([128, C], BF16, tag="xe0", name=f"xe0_{g}")
                nc.sync.dma_start(out=xe0[:], in_=xeT[g][0:128, :])
                xe1 = xep.tile([128, C], BF16, tag="xe1", name=f"xe1_{g}")
                nc.sync.dma_start(out=xe1[:], in_=xeT[g][128:256, :])

                # prefetch node-phase inputs for this group
                cols = slice(g * GRP, (g + 1) * GRP)
                xga = npl.tile([128, 512], BF16, tag="xga")
                nc.sync.dma_start(out=xga[:], in_=xTb[0:128, cols])
                xgb = npl.tile([128, 512], BF16, tag="xgb")
                nc.sync.dma_start(out=xgb[:], in_=xTb[128:256, cols])
                ytile = npl.tile([128, 8 * 512], BF16, tag="ytile")
                nc.sync.dma_start(out=ytile[:], in_=ydram[g])

                # edge supertiles
                pend = None          # deferred scatter work (prev supertile)

                def emit_scatter(work):
                    st0, L0, wgt0, oht0 = work
                    for lt in ([] if "scatter" in skip else range(L0)):
                        t = st0 + lt
                        col0 = starts[t]
                        wb = lt * 640
                        ob = lt * 4 * SPAN
                        last = (t == T - 1)
                        # bankP: [s0a|s1b_0] <- [A|B0] x oh_a0
                        nc.tensor.matmul(
                            banks[0][:, col0:col0 + SPAN],
                            lhsT=wgt0[:, wb:wb + 128],
                            rhs=oht0[:, ob:ob + SPAN],
                            start=False, stop=last, skip_group_check=True)
                        # bankQ: [s1b_1|s1b_2] <- [B1|B2] x oh_a0
                        nc.tensor.matmul(
                            banks[1][:, col0:col0 + SPAN],
                            lhsT=wgt0[:, wb + 128:wb + 256],
                            rhs=oht0[:, ob:ob + SPAN],
                            start=False, stop=last, skip_group_check=True)
                        # bankR_d: [s1a_d|s0b_d] <- [C|D_d] x oh_a1_d
                        for d in range(3):
                            nc.tensor.matmul(
                                banks[2 + d][:, col0:col0 + SPAN],
                                lhsT=wgt0[:, wb + 256 + 128 * d:
                                         wb + 384 + 128 * d],
                                rhs=oht0[:, ob + SPAN * (1 + d):
                                        ob + SPAN * (2 + d)],
                                start=False, stop=last,
                                skip_group_check=True)

                for st in range(0, T, STW):
                    L = min(STW, T - st)
                    oht = sp.tile([128, STW * 4 * SPAN], BF16, tag="oht")
                    nc.sync.dma_start(
                        out=oht[:, :L * 4 * SPAN],
                        in_=ohd[g][:, st * 4 * SPAN:(st + L) * 4 * SPAN])

                    # per-tile [h(256) | wp(256)] interleaved, bf16
                    hw_sb = sp.tile([128, STW * 512], BF16, tag="hw_sb")

                    # per-tile: lin1 + wp matmuls into one psum bank, then
                    # psum->sbuf copies split across scalar/gpsimd
                    for lt in range(L):
                        t = st + lt
                        hwp = wpp.tile([128, 512], F32, tag="wpb",
                                       name=f"hwp_{g}_{t}")
                        if "lin1" not in skip:
                            nc.tensor.matmul(hwp[:, 0:128],
                                             lhsT=xe0[:, 128 * t:128 * t + 128],
                                             rhs=wblk1[:],
                                             start=True, stop=True)
                            nc.tensor.matmul(hwp[:, 128:256],
                                             lhsT=xe1[:, 128 * t:128 * t + 128],
                                             rhs=wblk2[:],
                                             start=True, stop=True)
                        if "wp" not in skip:
                            hb = 32 * (t % 3)
                            nc.tensor.matmul(hwp[:, 256:512],
                                             lhsT=hid[hb:hb + 8,
                                                      128 * (t // 3):
                                                      128 * (t // 3) + 128],
                                             rhs=wbig[hb:hb + 8, :],
                                             start=True, stop=True)
                        # psum->sbuf copies (only scalar/DVE may read PSUM);
                        # one [128,512] op per bank, alternating engines
                        dst = hw_sb[:, 512 * lt:512 * lt + 512]
                        if cpcnt[0] % 5 in (0, 2, 4):
                            nc.scalar.activation(dst, hwp[:], ACT_COPY)
                        else:
                            nc.vector.tensor_copy(out=dst, in_=hwp[:])
                        cpcnt[0] += 1

                    # wg products: [A|B0|B1|B2|C|D0|C|D1|C|D2] per tile
                    # layout: 640 cols/tile: A(64) B_d(192) [C|D_d](3x128)
                    wgt = sp.tile([128, STW * 640], BF16, tag="wgt", bufs=2)
                    if "wg" in skip:
                        pass
                    else:
                      # A = w1*g0 (gpsimd, sbuf only)
                      nc.gpsimd.tensor_tensor(
                        out=_apv(wgt[:], 0, [[640, L], [1, 64]]),
                        in0=_apv(hw_sb[:], 256, [[512, L], [1, 64]]),
                        in1=_apv(hw_sb[:], 0, [[512, L], [1, 64]]),
                        op=MULT)
                      # B_d = w3*g1_d
                      nc.vector.tensor_tensor(
                        out=_apv(wgt[:], 64, [[640, L], [64, 3], [1, 64]]),
                        in0=_apv(hw_sb[:], 256 + 64,
                                 [[512, L], [0, 3], [1, 64]]),
                        in1=_apv(hw_sb[:], 64, [[512, L], [64, 3], [1, 64]]),
                        op=MULT)
                      # C = w2*g0 (replicated 3x at 256+128d)
                      nc.vector.tensor_tensor(
                        out=_apv(wgt[:], 256, [[640, L], [128, 3], [1, 64]]),
                        in0=_apv(hw_sb[:], 256 + 128,
                                 [[512, L], [0, 3], [1, 64]]),
                        in1=_apv(hw_sb[:], 0, [[512, L], [0, 3], [1, 64]]),
                        op=MULT)
                      # D_d = w4'*g1_d (at 320+128d)
                      nc.vector.tensor_tensor(
                        out=_apv(wgt[:], 320, [[640, L], [128, 3], [1, 64]]),
                        in0=_apv(hw_sb[:], 256 + 192,
                                 [[512, L], [0, 3], [1, 64]]),
                        in1=_apv(hw_sb[:], 64, [[512, L], [64, 3], [1, 64]]),
                        op=MULT)

                    # compact scatter matmuls (deferred one supertile so the
                    # PE can pipeline lin1/wp of st+1 with wg of st)
                    if pend is not None:
                        emit_scatter(pend)
                    pend = (st, L, wgt, oht)
                if pend is not None:
                    emit_scatter(pend)

                # drain banks -> sts (bf16)
                sts = []
                for k in range(5):
                    stile = stsp.tile([128, 512], BF16, tag=f"sts{k}",
                                      name=f"sts{k}_{g}")
                    if k % 2 == 0:
                        nc.scalar.activation(stile[:], banks[k][:], ACT_COPY)
                    else:
                        nc.vector.tensor_copy(out=stile[:], in_=banks[k][:])
                    sts.append(stile)
                stsP, stsQ, stsR = sts[0], sts[1], sts[2:5]

                # ---------------- node phase for this group ----------------
                if "node" in skip:
                    outa = npl.tile([128, 512], BF16, tag="outa")
                    nc.vector.tensor_copy(out=outa[:], in_=xga[:])
                    outb = npl.tile([128, 512], BF16, tag="outb")
                    nc.vector.tensor_copy(out=outb[:], in_=xgb[:])
                else:
                    # up0 = W20^T s0 + sc0   [scalars|gates, 512]
                    up0 = bkp.tile([128, 512], F32, tag="bank0",
                                   name=f"up0_{g}")
                    sRt = npl.tile([128, 512], BF16, tag="sRt")
                    nc.vector.tensor_add(out=sRt[:], in0=stsR[0][:],
                                         in1=stsR[1][:])
                    nc.vector.tensor_add(out=sRt[:], in0=sRt[:],
                                         in1=stsR[2][:])
                    nc.tensor.matmul(up0[:], lhsT=w20a_p[:],
                                     rhs=stsP[:], start=True, stop=False)
                    nc.tensor.matmul(up0[:], lhsT=w20b_p[:],
                                     rhs=sRt[:],
                                     start=False, stop=False)
                    nc.tensor.matmul(up0[:], lhsT=wsc0a[:],
                                     rhs=ytile[:, 0:512],
                                     start=False, stop=False)
                    nc.tensor.matmul(up0[:], lhsT=wsc0b[:],
                                     rhs=ytile[:, 512:1024],
                                     start=False, stop=True)

                    # up1: d0 rows 0:64, d1 rows 64:128 of up1a; d2 in up1b
                    up1a = bkp.tile([128, 512], F32, tag="bank1",
                                     name=f"up1a_{g}")
                    for d in (0, 1):
                        rows = slice(64 * d, 64 * d + 64)
                        if d == 0:
                            s1b_src, s1b_w = stsP[:], w21b_b[:]
                        else:
                            s1b_src, s1b_w = stsQ[:], w21b_t[:]
                        nc.tensor.matmul(up1a[rows, :], lhsT=w21a_t[:],
                                         rhs=stsR[d][:],
                                         start=True, stop=False)
                        nc.tensor.matmul(up1a[rows, :], lhsT=s1b_w, rhs=s1b_src,
                                         start=False, stop=False)
                        yo = 1024 * (1 + d)
                        nc.tensor.matmul(up1a[rows, :], lhsT=wsc1a[:],
                                         rhs=ytile[:, yo:yo + 512],
                                         start=False, stop=False)
                        nc.tensor.matmul(up1a[rows, :], lhsT=wsc1b[:],
                                         rhs=ytile[:, yo + 512:yo + 1024],
                                         start=False, stop=True)

                    # gate scalars/gates -> bf16
                    t0s = npl.tile([128, 512], BF16, tag="t0s")
                    nc.scalar.activation(t0s[:], up0[:], ACT_SILU)

                    up1b = wpp.tile([64, 512], F32, tag="wpb",
                                    name=f"up1b_{g}")
                    nc.tensor.matmul(up1b[:], lhsT=w21a_t[:],
                                     rhs=stsR[2][:], start=True, stop=False)
                    nc.tensor.matmul(up1b[:], lhsT=w21b_b[:],
                                     rhs=stsQ[:], start=False, stop=False)
                    nc.tensor.matmul(up1b[:], lhsT=wsc1a[:],
                                     rhs=ytile[:, 3072:3584],
                                     start=False, stop=False)
                    nc.tensor.matmul(up1b[:], lhsT=wsc1b[:],
                                     rhs=ytile[:, 3584:4096],
                                     start=False, stop=True)

                    # assemble: vectors = gates*t1 (t1 from psum); resnet add
                    outa = npl.tile([128, 512], BF16, tag="outa")
                    nc.vector.tensor_add(out=outa[0:64, :], in0=t0s[0:64, :],
                                         in1=xga[0:64, :])
                    nc.vector.tensor_tensor(out=outa[64:128, :],
                                            in0=t0s[64:128, :],
                                            in1=up1a[0:64, :], op=MULT)
                    nc.vector.tensor_add(out=outa[64:128, :], in0=outa[64:128, :],
                                         in1=xga[64:128, :])
                    outb = npl.tile([128, 512], BF16, tag="outb")
                    nc.vector.tensor_tensor(out=outb[0:64, :],
                                            in0=t0s[64:128, :],
                                            in1=up1a[64:128, :], op=MULT)
                    nc.vector.tensor_add(out=outb[0:64, :], in0=outb[0:64, :],
                                         in1=xgb[0:64, :])
                    nc.vector.tensor_tensor(out=outb[64:128, :],
                                            in0=t0s[64:128, :],
                                            in1=up1b[:], op=MULT)
                    nc.vector.tensor_add(out=outb[64:128, :], in0=outb[64:128, :],
                                         in1=xgb[64:128, :])

                nc.sync.dma_start(out=outT[0:128, cols], in_=outa[:])
                nc.sync.dma_start(out=outT[128:256, cols], in_=outb[:])

    nc.compile()
    return nc


# ---------------------------------------------------------------- host prep
def _pack_group(cols, T):
    """Greedy pack of sorted dst-cols into T tiles on the uniform grid.
    Returns per-tile edge index lists (positions into cols) or None."""
    starts = _grid_starts(T)
    res = []
    j, nE = 0, len(cols)
    for t in range(T):
        lo, hi = starts[t], starts[t] + SPAN
        tl = []
        while j < nE and len(tl) < 128 and cols[j] < hi:
            if cols[j] < lo:
                return None
            tl.append(j)
            j += 1
        res.append(tl)
    if j < nE:
        return None
    return res


def _host_prep(node_feats, node_attrs, edge_attrs, edge_embedding,
               W_lin1_0, W_lin1_1, W_mlp1, W_mlp2,
               W_lin2_0, W_lin2_1, W_sc0, W_sc1, edge_index):
    inv = 1.0 / np.sqrt(MUL)
    inv_e = 1.0 / np.sqrt(EDIM)
    inv2 = 1.0 / np.sqrt(2 * MUL)
    inv_n = 1.0 / np.sqrt(AVG_NEIGH)
    inv_sc = 1.0 / np.sqrt(MUL * NZ)

    # channel permutation: ours = [x0(64) | x1 d-major(192)]
    gidx = np.empty(256, np.int64)
    gidx[:64] = np.arange(64)
    for d in range(3):
        for u in range(64):
            gidx[64 + 64 * d + u] = 64 + 3 * u + d

    # permuted node feats in bf16 (row N = zero pad row for empty slots)
    xg_pad = np.zeros((N + 1, 256), np.float32)
    xg_pad[:N] = node_feats[:, gidx]
    xg_pad_b = xg_pad.astype(NP_BF16)

    xgf = np.zeros((NP_PAD, 256), np.float32)
    xgf[:N] = node_feats[:, gidx]
    xT = np.ascontiguousarray(xgf.T)
    xTf = xT.astype(NP_BF16)

    attT = np.zeros((NZ, NP_PAD), np.float32)
    attT[:, :N] = node_attrs.T.astype(np.float32)

    # ---- edge sorting and per-(core,group) packing
    src = edge_index[0].astype(np.int64)
    dst = edge_index[1].astype(np.int64)
    order = np.argsort(dst, kind="stable")
    src_s, dst_s = src[order], dst[order]
    ea_s = edge_attrs[order].astype(np.float32)
    emb_s = edge_embedding[order].astype(np.float32)

    bounds = np.searchsorted(dst_s, np.arange(0, NP_PAD + 1, GRP))
    all_cols = []
    T = 2
    for c in range(CORES):
        for g in range(NG):
            gi = c * NG + g
            s, e = bounds[gi], bounds[gi + 1]
            cols = (dst_s[s:e] - gi * GRP).astype(int)
            all_cols.append(cols)
            Tg = max(1, int(np.ceil(len(cols) / 128)))
            while Tg < 96 and _pack_group(cols, Tg) is None:
                Tg += 1
            T = max(T, Tg)
    T = T + (T % 2)  # even
    while any(_pack_group(cols, T) is None for cols in all_cols):
        T += 2

    C = T * 128

    per_core = []
    for c in range(CORES):
        slot_src = np.full((NG, C), N, np.int64)   # default -> zero row
        oh = np.zeros((NG, T, 128, 4 * SPAN), np.float32)
        embw = np.zeros((NG, EDIM, C), np.float32)
        starts = _grid_starts(T)
        for g in range(NG):
            gi = c * NG + g
            s = bounds[gi]
            cols = all_cols[gi]
            pk = _pack_group(cols, T)
            assert pk is not None
            for t, tl in enumerate(pk):
                if not tl:
                    continue
                idx = np.asarray(tl, np.int64)
                p = np.arange(len(tl))
                slot = t * 128 + p
                sn = src_s[s + idx]
                slot_src[g, slot] = sn
                embw[g, :, slot] = emb_s[s + idx]
                cc = cols[idx] - starts[t]
                oh[g, t, p, cc] = ea_s[s + idx, 0]               # oh*a0
                for d in range(3):
                    oh[g, t, p, SPAN * (1 + d) + cc] = ea_s[s + idx, 1 + d]
        # pre-gathered x per slot, channel-major [NG, 256, C]
        xe = xg_pad_b[slot_src.reshape(-1)]          # [NG*C, 256] bf16
        xe_dev = np.ascontiguousarray(
            xe.reshape(NG, C, 256).transpose(0, 2, 1))
        # device layout [NG, 128, T*4*SPAN]
        oh_dev = oh.transpose(0, 2, 1, 3).reshape(NG, 128, T * 4 * SPAN)
        # embd blockdiag3 layout: [NG, 32*(t%3)+j, 128*(t//3)+n] for
        # slot t*128 + n (rows 8:32 of each 32-block and 96:128 zero)
        NT3 = (T + 2) // 3
        embp = np.zeros((NG, EDIM, NT3 * 384), np.float32)
        embp[:, :, :C] = embw
        embp = embp.reshape(NG, EDIM, NT3, 3, 128)
        emb2 = np.zeros((NG, 4, 32, NT3, 128), np.float32)
        emb2[:, :3, :EDIM] = embp.transpose(0, 3, 1, 2, 4)
        emb2 = emb2.reshape(NG, 128, NT3 * 128)
        per_core.append(dict(
            xeT=xe_dev,
            ohd=np.ascontiguousarray(oh_dev).astype(NP_BF16),
            embd=np.ascontiguousarray(emb2).astype(NP_BF16),
        ))

    # ---- weights
    W10s = (W_lin1_0 * inv).astype(np.float32)
    W11s = (W_lin1_1 * inv).astype(np.float32)
    wblk1 = np.zeros((128, 128), np.float32)
    wblk1[:64, :64] = W10s
    wblk1[64:, 64:] = W11s
    wblk2 = np.zeros((128, 128), np.float32)
    wblk2[:64, :64] = W11s
    wblk2[64:, 64:] = W11s
    wm1s = (W_mlp1 * inv_e).astype(np.float32)
    wm1 = np.zeros((128, 128), np.float32)
    for q in range(3):
        wm1[32 * q:32 * q + EDIM, 32 * q:32 * q + EDIM] = wm1s
    wm1 = wm1.astype(NP_BF16)
    w1 = W_mlp2[:, 0:64]
    w2 = W_mlp2[:, 64:128]
    w3 = W_mlp2[:, 128:192]
    w4 = W_mlp2[:, 192:256]
    # wp cols: [w1 | w3 | w2 | w4'], replicated in 4 partition quadrants
    wbig8 = (np.concatenate([w1, w3, w2, w4 * INV_SQRT3], axis=1)
             * inv_e).astype(np.float32)
    wbig = np.zeros((128, 256), np.float32)
    for q in range(3):
        wbig[32 * q:32 * q + EDIM] = wbig8
    wbig = wbig.astype(NP_BF16)
    w20s = (W_lin2_0 * inv2 * inv_n).astype(np.float32)
    w21s = (W_lin2_1 * inv2 * inv_n).astype(np.float32)
    z64x128 = np.zeros((64, 128), np.float32)
    z64x64 = np.zeros((64, 64), np.float32)
    w20a_p = np.concatenate([w20s[0:64], z64x128]).astype(NP_BF16)
    w20b_p = np.concatenate([z64x128, w20s[64:128]]).astype(NP_BF16)
    w21a_t = np.concatenate([w21s[0:64], z64x64]).astype(NP_BF16)
    w21b_t = np.concatenate([w21s[64:128], z64x64]).astype(NP_BF16)
    w21b_b = np.concatenate([z64x64, w21s[64:128]]).astype(NP_BF16)
    wsc0z = (np.transpose(W_sc0, (1, 0, 2)).reshape(NZ * MUL, 2 * MUL)
             * inv_sc).astype(NP_BF16)
    wsc1z = (np.transpose(W_sc1, (1, 0, 2)).reshape(NZ * MUL, MUL)
             * inv_sc).astype(NP_BF16)
    wsc0 = np.stack([wsc0z[:128], wsc0z[128:]])
    wsc1 = np.stack([wsc1z[:128], wsc1z[128:]])

    shared = dict(wblk1=wblk1.astype(NP_BF16),
                  wblk2=wblk2.astype(NP_BF16), wm1=wm1, wbig=wbig,
                  w20a_p=w20a_p, w20b_p=w20b_p, w21a_t=w21a_t,
                  w21b_t=w21b_t, w21b_b=w21b_b, wsc0=wsc0, wsc1=wsc1)
    in_maps = []
    for c in range(CORES):
        m = dict(shared)
        csl = slice(c * NPC, (c + 1) * NPC)
        m["xTb"] = np.ascontiguousarray(xTf[:, csl])
        # host-precomputed self-connection inputs y = x_block * z_attr
        # block (k, h): cols 512*(2k+h), rows z*64+u (z = 2h + p//64)
        ycore = np.empty((128, NPC, 8), np.float32)
        xTc = xT[:, csl]
        atc = attT[:, csl]
        for k in range(4):
            xk = xTc[64 * k:64 * k + 64]
            for h in range(2):
                ycore[0:64, :, 2 * k + h] = xk * atc[2 * h]
                ycore[64:128, :, 2 * k + h] = xk * atc[2 * h + 1]
        yd = ycore.reshape(128, NG, 512, 8).transpose(1, 0, 3, 2).reshape(
            NG, 128, 8 * 512)
        m["ydram"] = np.ascontiguousarray(yd).astype(NP_BF16)
        m.update(per_core[c])
        in_maps.append(m)
    return T, in_maps, gidx


_PROGRAM_CACHE = {}


def kernel(**inputs):
    global LAST_RESULT
    _install_profile_hook()

    args = {k: np.asarray(v) for k, v in inputs.items()}
    T, in_maps, gidx = _host_prep(
        args["node_feats"].astype(np.float32),
        args["node_attrs"].astype(np.float32),
        args["edge_attrs"].astype(np.float32),
        args["edge_embedding"].astype(np.float32),
        args["W_lin1_0"].astype(np.float32),
        args["W_lin1_1"].astype(np.float32),
        args["W_mlp1"].astype(np.float32),
        args["W_mlp2"].astype(np.float32),
        args["W_lin2_0"].astype(np.float32),
        args["W_lin2_1"].astype(np.float32),
        args["W_sc0"].astype(np.float32),
        args["W_sc1"].astype(np.float32),
        args["edge_index"])

    if T not in _PROGRAM_CACHE:
        _PROGRAM_CACHE[T] = _build_program(T)
    nc = _PROGRAM_CACHE[T]

    trace = bool(int(os.environ.get("BASS_TRACE", "0")))
    res = run_bass_kernel_spmd(nc, in_maps, core_ids=list(range(CORES)),
                               trace=trace)
    LAST_RESULT = res

    outT = np.concatenate(
        [res.results[c]["outT"].astype(np.float32) for c in range(CORES)],
        axis=1)                            # [256, NP_PAD]
    full = outT.T[:N]                      # [N, 256] in our channel order
    out = np.empty((N, 256), np.float32)
    out[:, gidx] = full
    return out


# revision 19
# speedup vs baseline: 1.2013x; 1.0675x over previous
"""Trainium2 Bass kernel for PointConv-style e3nn message passing.

Self-contained: builds + runs an 8-core SPMD Bass kernel via
bass_utils.run_bass_kernel_spmd, accepting FULL inputs and returning the
FULL output.

Design (v3):
- Nodes padded to 20480, split 8 ways (2560/core); edges sorted by dst and
  assigned to the core owning the destination.
- Per core, destinations are processed in 5 groups of 512 nodes. Edges of a
  group are packed into T 128-edge tiles on a uniform column grid (each tile
  owns a baked 32-column window of the group's 512 psum columns), so the
  scatter-add becomes per-tile compact one-hot matmuls into 5 psum banks.
- The a0/a1 spherical-harmonic factors are folded into host-prescaled
  one-hots (oh*a0, oh*a1_d), so the device only forms the w*g products.
- v3 change vs v2: no replicated h-table and no device dma_gather. The host
  pre-gathers x[src] per edge slot (channel-major, slot order) and the
  device computes h = lin1(x) per 128-edge tile with two blockdiag matmuls
  directly into psum, alongside the radial tp-weight matmul.
"""

import os
import sys
import types
import ctypes

import numpy as np

import concourse.bass as bass
import concourse.bacc as bacc
import concourse.tile as tile
from concourse import mybir
from concourse.bass import AP
from concourse.bass_utils import run_bass_kernel_spmd

# ---------------------------------------------------------------- constants
N = 20000
E = 160000
MUL = 64
EDIM = 8
NZ = 4
AVG_NEIGH = 8.0
INV_SQRT3 = float(1.0 / np.sqrt(3.0))

CORES = 8
NP_PAD = 20480            # padded node count
NPC = NP_PAD // CORES     # 2560 nodes per core
GRP = 512                 # nodes per scatter group (psum bank width)
NG = NPC // GRP           # 5 groups per core
SPAN = 32                 # onehot column window per edge tile
BACK = 8                  # grid look-back
STW = 8                   # supertile width (tiles per DVE batch)

F32 = mybir.dt.float32
BF16 = mybir.dt.bfloat16
I32 = mybir.dt.int32
NP_BF16 = mybir.dt.np(mybir.dt.bfloat16)

LAST_RESULT = None


# ------------------------------------------------------- axon profile hook
def _install_profile_hook():
    """Make trace=True / BASS_TRACE=1 work under axon (degrades silently)."""
    if "antenv.axon_hooks" in sys.modules:
        return
    try:
        try:
            from trn_agent_boot.trn_boot import _ntff_profile_via_ctypes
        except ImportError:
            sys.path.insert(0, "/root/.axon_site")
            from trn_agent_boot.trn_boot import _ntff_profile_via_ctypes
        so_path = "/opt/axon/libaxon_pjrt.so"
        lib = ctypes.CDLL(so_path)
        if not hasattr(lib, "axon_start_nrt_profile"):
            return
        hook = _ntff_profile_via_ctypes(so_path)
        mod = types.ModuleType("antenv.axon_hooks")
        state = {"hook": hook}
        mod.set_axon_ntff_profile_hook = lambda h: state.__setitem__("hook", h)
        mod.get_axon_ntff_profile_hook = lambda: state["hook"]
        sys.modules["antenv.axon_hooks"] = mod
        import antenv
        antenv.axon_hooks = mod
    except Exception:
        pass


# ----------------------------------------------- tile-exit drain workaround
def _patch_tile_drain():
    """This toolchain's walrus rejects >1 sem wait on a Drain; hang the exit
    waits on a NoOp chain instead (bacc's generate_event_semaphores then
    legalises them)."""
    from concourse.vector_clock import ScopedClock

    def _drain_and_barrier(self, tick_clock, wait_clock):
        nop_inst = self.nc.sync.nop(nofuse=True, hint="tile_exit_wait")
        wait_clock.add_sem_waits(
            nop_inst.ins, ScopedClock({None: tick_clock.global_clock})
        )
        self.nc.sync.drain()
        self.nc.all_engine_barrier()
        assert self.sems is not None
        popped = self.nc._tile_sem_poison_stack.pop()
        assert popped is self._sem_poison
        self.nc.clear_and_free_semaphores(list(self.sems.allocated().values()))
        self.nc.all_engine_barrier()

    tile.TileContext._drain_and_barrier = _drain_and_barrier


_patch_tile_drain()


def _grid_starts(T):
    return [max(0, min(int(round(i * GRP / T)) - BACK, GRP - SPAN))
            for i in range(T)]


def _apv(base_ap, col_off, dims):
    """AP view of a 2D sbuf/psum tile: partitions x custom free dims."""
    pstep, pcount = base_ap.ap[0]
    return AP(base_ap.tensor, base_ap.offset + col_off,
              [[pstep, pcount]] + dims)


# ---------------------------------------------------------------- program
def _build_program(T):
    """Build the SPMD Bass program for T edge tiles per 512-node group."""
    C = T * 128               # edge slots per group
    skip = set(os.environ.get("BASS_SKIP", "").split(","))

    nc = bacc.Bacc(num_swdge_queues=4)

    # inputs (per core)
    xeT = nc.dram_tensor("xeT", [NG, 256, C], BF16, kind="ExternalInput")
    xTb = nc.dram_tensor("xTb", [256, NPC], BF16, kind="ExternalInput")
    ydram = nc.dram_tensor("ydram", [NG, 128, 8 * 512], BF16,
                           kind="ExternalInput")
    ohd = nc.dram_tensor("ohd", [NG, 128, T * 4 * SPAN], BF16,
                         kind="ExternalInput")
    NT3 = (T + 2) // 3
    CB = NT3 * 128
    embd = nc.dram_tensor("embd", [NG, 128, CB], BF16, kind="ExternalInput")
    wblk1_d = nc.dram_tensor("wblk1", [128, 128], BF16, kind="ExternalInput")
    wblk2_d = nc.dram_tensor("wblk2", [128, 128], BF16, kind="ExternalInput")
    wm1_d = nc.dram_tensor("wm1", [128, 128], BF16, kind="ExternalInput")
    wbig_d = nc.dram_tensor("wbig", [128, 256], BF16, kind="ExternalInput")
    w20a_d = nc.dram_tensor("w20a_p", [128, 128], BF16, kind="ExternalInput")
    w20b_d = nc.dram_tensor("w20b_p", [128, 128], BF16, kind="ExternalInput")
    w21at_d = nc.dram_tensor("w21a_t", [128, 64], BF16, kind="ExternalInput")
    w21bt_d = nc.dram_tensor("w21b_t", [128, 64], BF16, kind="ExternalInput")
    w21bb_d = nc.dram_tensor("w21b_b", [128, 64], BF16, kind="ExternalInput")
    wsc0_d = nc.dram_tensor("wsc0", [2, 128, 128], BF16, kind="ExternalInput")
    wsc1_d = nc.dram_tensor("wsc1", [2, 128, 64], BF16, kind="ExternalInput")
    outT = nc.dram_tensor("outT", [256, NPC], BF16, kind="ExternalOutput")

    ACT_SILU = mybir.ActivationFunctionType.Silu
    ACT_COPY = mybir.ActivationFunctionType.Copy
    MULT = mybir.AluOpType.mult

    starts = _grid_starts(T)

    with tile.TileContext(nc) as tc:
        with (
            tc.tile_pool(name="const", bufs=1) as cp,
            tc.tile_pool(name="grp", bufs=2) as gp,
            tc.tile_pool(name="hid5", bufs=5) as hp5,
            tc.tile_pool(name="xe", bufs=2) as xep,
            tc.tile_pool(name="st", bufs=3) as sp,
            tc.tile_pool(name="sts", bufs=2) as stsp,
            tc.tile_pool(name="node", bufs=2) as npl,
            tc.tile_pool(name="bank", bufs=1, space="PSUM") as bkp,
            tc.tile_pool(name="wps", bufs=3, space="PSUM") as wpp,
        ):
            # ---- constants
            wblk1 = cp.tile([128, 128], BF16)
            nc.sync.dma_start(out=wblk1[:], in_=wblk1_d[:])
            wblk2 = cp.tile([128, 128], BF16)
            nc.sync.dma_start(out=wblk2[:], in_=wblk2_d[:])
            wm1 = cp.tile([128, 128], BF16)
            nc.sync.dma_start(out=wm1[:], in_=wm1_d[:])
            wbig = cp.tile([128, 256], BF16)
            nc.sync.dma_start(out=wbig[:], in_=wbig_d[:])
            w20a_p = cp.tile([128, 128], BF16)
            nc.sync.dma_start(out=w20a_p[:], in_=w20a_d[:])
            w20b_p = cp.tile([128, 128], BF16)
            nc.sync.dma_start(out=w20b_p[:], in_=w20b_d[:])
            w21a_t = cp.tile([128, 64], BF16)
            nc.sync.dma_start(out=w21a_t[:], in_=w21at_d[:])
            w21b_t = cp.tile([128, 64], BF16)
            nc.sync.dma_start(out=w21b_t[:], in_=w21bt_d[:])
            w21b_b = cp.tile([128, 64], BF16)
            nc.sync.dma_start(out=w21b_b[:], in_=w21bb_d[:])
            wsc0a = cp.tile([128, 128], BF16)
            nc.sync.dma_start(out=wsc0a[:], in_=wsc0_d[0])
            wsc0b = cp.tile([128, 128], BF16)
            nc.sync.dma_start(out=wsc0b[:], in_=wsc0_d[1])
            wsc1a = cp.tile([128, 64], BF16)
            nc.sync.dma_start(out=wsc1a[:], in_=wsc1_d[0])
            wsc1b = cp.tile([128, 64], BF16)
            nc.sync.dma_start(out=wsc1b[:], in_=wsc1_d[1])
            zeros = cp.tile([128, 512], BF16)
            nc.vector.memset(zeros[:], 0.0)

            # ============ radial MLP for all groups (independent of x) ====
            # blockdiag3(Wm1) packing: 3 tiles share partitions at 32-part
            # spacing (p = 32*(t%3)+j; base partition must be 0/32/64).
            # hid_g[32*(t%3)+u, 128*(t//3)+n] is the tp-weight for slot
            # t*128 + n.
            hids = []
            for g in range(NG):
                embt = gp.tile([128, CB], BF16, tag="embt")
                nc.sync.dma_start(out=embt[:], in_=embd[g])
                hid = hp5.tile([128, CB], BF16, tag="hid", name=f"hid{g}")
                for c0 in range(0, CB, 512):
                    sz = min(512, CB - c0)
                    hp = wpp.tile([128, 512], F32, tag="wpb")
                    nc.tensor.matmul(hp[:, :sz], lhsT=wm1[:],
                                     rhs=embt[:, c0:c0 + sz],
                                     start=True, stop=True)
                    nc.scalar.activation(hid[:, c0:c0 + sz], hp[:, :sz],
                                         ACT_SILU)
                hids.append(hid)

            # scatter psum banks (reused across groups)
            def bank_tiles():
                return [bkp.tile([128, 512], F32, tag=f"bank{k}",
                                 name=f"bank{k}") for k in range(5)]

            # ================= per-group edge + node phases ================
            cpcnt = [0]
            for g in range(NG):
                banks = bank_tiles()
                if "memset" not in skip:
                    for k in range(5):
                        if k % 2 == 0:
                            nc.vector.memset(banks[k][:], 0.0)
                        else:
                            nc.scalar.activation(banks[k][:], zeros[:],
                                                 ACT_COPY)

                hid = hids[g]

                # per-slot x inputs (channel-major, slot order)
                xe0 = xep.tile([128, C], BF16, tag="xe0", name=f"xe0_{g}")
                nc.sync.dma_start(out=xe0[:], in_=xeT[g][0:128, :])
                xe1 = xep.tile([128, C], BF16, tag="xe1", name=f"xe1_{g}")
                nc.sync.dma_start(out=xe1[:], in_=xeT[g][128:256, :])

                # prefetch node-phase inputs for this group
                cols = slice(g * GRP, (g + 1) * GRP)
                xga = npl.tile([128, 512], BF16, tag="xga")
                nc.sync.dma_start(out=xga[:], in_=xTb[0:128, cols])
                xgb = npl.tile([128, 512], BF16, tag="xgb")
                nc.sync.dma_start(out=xgb[:], in_=xTb[128:256, cols])
                ytile = npl.tile([128, 8 * 512], BF16, tag="ytile")
                nc.sync.dma_start(out=ytile[:], in_=ydram[g])

                # edge supertiles
                pend = None          # deferred scatter work (prev supertile)

                def emit_scatter(work):
                    st0, L0, wgt0, oht0 = work
                    for lt in ([] if "scatter" in skip else range(L0)):
                        t = st0 + lt
                        col0 = starts[t]
                        wb = lt * 640
                        ob = lt * 4 * SPAN
                        last = (t == T - 1)
                        # bankP: [s0a|s1b_0] <- [A|B0] x oh_a0
                        nc.tensor.matmul(
                            banks[0][:, col0:col0 + SPAN],
                            lhsT=wgt0[:, wb:wb + 128],
                            rhs=oht0[:, ob:ob + SPAN],
                            start=False, stop=last, skip_group_check=True)
                        # bankQ: [s1b_1|s1b_2] <- [B1|B2] x oh_a0
                        nc.tensor.matmul(
                            banks[1][:, col0:col0 + SPAN],
                            lhsT=wgt0[:, wb + 128:wb + 256],
                            rhs=oht0[:, ob:ob + SPAN],
                            start=False, stop=last, skip_group_check=True)
                        # bankR_d: [s1a_d|s0b_d] <- [C|D_d] x oh_a1_d
                        for d in range(3):
                            nc.tensor.matmul(
                                banks[2 + d][:, col0:col0 + SPAN],
                                lhsT=wgt0[:, wb + 256 + 128 * d:
                                         wb + 384 + 128 * d],
                                rhs=oht0[:, ob + SPAN * (1 + d):
                                        ob + SPAN * (2 + d)],
                                start=False, stop=last,
                                skip_group_check=True)

                for st in range(0, T, STW):
                    L = min(STW, T - st)
                    oht = sp.tile([128, STW * 4 * SPAN], BF16, tag="oht")
                    nc.sync.dma_start(
                        out=oht[:, :L * 4 * SPAN],
                        in_=ohd[g][:, st * 4 * SPAN:(st + L) * 4 * SPAN])

                    # per-tile [h(256) | wp(256)] interleaved, bf16
                    hw_sb = sp.tile([128, STW * 512], BF16, tag="hw_sb")

                    # per-tile: lin1 + wp matmuls into one psum bank, then
                    # psum->sbuf copies split across scalar/gpsimd
                    for lt in range(L):
                        t = st + lt
                        hwp = wpp.tile([128, 512], F32, tag="wpb",
                                       name=f"hwp_{g}_{t}")
                        if "lin1" not in skip:
                            nc.tensor.matmul(hwp[:, 0:128],
                                             lhsT=xe0[:, 128 * t:128 * t + 128],
                                             rhs=wblk1[:],
                                             start=True, stop=True)
                            nc.tensor.matmul(hwp[:, 128:256],
                                             lhsT=xe1[:, 128 * t:128 * t + 128],
                                             rhs=wblk2[:],
                                             start=True, stop=True)
                        if "wp" not in skip:
                            hb = 32 * (t % 3)
                            nc.tensor.matmul(hwp[:, 256:512],
                                             lhsT=hid[hb:hb + 32,
                                                      128 * (t // 3):
                                                      128 * (t // 3) + 128],
                                             rhs=wbig[hb:hb + 32, :],
                                             start=True, stop=True)
                        # psum->sbuf copies (only scalar/DVE may read PSUM);
                        # one [128,512] op per bank, alternating engines
                        dst = hw_sb[:, 512 * lt:512 * lt + 512]
                        if cpcnt[0] % 4 != 3:
                            nc.scalar.activation(dst, hwp[:], ACT_COPY)
                        else:
                            nc.vector.tensor_copy(out=dst, in_=hwp[:])
                        cpcnt[0] += 1

                    # wg products: [A|B0|B1|B2|C|D0|C|D1|C|D2] per tile
                    # layout: 640 cols/tile: A(64) B_d(192) [C|D_d](3x128)
                    wgt = sp.tile([128, STW * 640], BF16, tag="wgt", bufs=3)
                    if "wg" in skip:
                        pass
                    else:
                      # A = w1*g0 (gpsimd, sbuf only)
                      nc.gpsimd.tensor_tensor(
                        out=_apv(wgt[:], 0, [[640, L], [1, 64]]),
                        in0=_apv(hw_sb[:], 256, [[512, L], [1, 64]]),
                        in1=_apv(hw_sb[:], 0, [[512, L], [1, 64]]),
                        op=MULT)
                      # B_d = w3*g1_d
                      nc.vector.tensor_tensor(
                        out=_apv(wgt[:], 64, [[640, L], [64, 3], [1, 64]]),
                        in0=_apv(hw_sb[:], 256 + 64,
                                 [[512, L], [0, 3], [1, 64]]),
                        in1=_apv(hw_sb[:], 64, [[512, L], [64, 3], [1, 64]]),
                        op=MULT)
                      # C = w2*g0 (replicated 3x at 256+128d)
                      nc.vector.tensor_tensor(
                        out=_apv(wgt[:], 256, [[640, L], [128, 3], [1, 64]]),
                        in0=_apv(hw_sb[:], 256 + 128,
                                 [[512, L], [0, 3], [1, 64]]),
                        in1=_apv(hw_sb[:], 0, [[512, L], [0, 3], [1, 64]]),
                        op=MULT)
                      # D_d = w4'*g1_d (at 320+128d)
                      nc.vector.tensor_tensor(
                        out=_apv(wgt[:], 320, [[640, L], [128, 3], [1, 64]]),
                        in0=_apv(hw_sb[:], 256 + 192,
                                 [[512, L], [0, 3], [1, 64]]),
                        in1=_apv(hw_sb[:], 64, [[512, L], [64, 3], [1, 64]]),
                        op=MULT)

                    # compact scatter matmuls (deferred one supertile so the
                    # PE can pipeline lin1/wp of st+1 with wg of st)
                    if pend is not None:
                        emit_scatter(pend)
                    pend = (st, L, wgt, oht)
                if pend is not None:
                    emit_scatter(pend)

                # drain banks -> sts (bf16)
                sts = []
                for k in range(5):
                    stile = stsp.tile([128, 512], BF16, tag=f"sts{k}",
                                      name=f"sts{k}_{g}")
                    if k != 1:
                        nc.scalar.activation(stile[:], banks[k][:], ACT_COPY)
                    else:
                        nc.vector.tensor_copy(out=stile[:], in_=banks[k][:])
                    sts.append(stile)
                stsP, stsQ, stsR = sts[0], sts[1], sts[2:5]

                # ---------------- node phase for this group ----------------
                if "node" in skip:
                    outa = npl.tile([128, 512], BF16, tag="outa")
                    nc.vector.tensor_copy(out=outa[:], in_=xga[:])
                    outb = npl.tile([128, 512], BF16, tag="outb")
                    nc.vector.tensor_copy(out=outb[:], in_=xgb[:])
                else:
                    # up0 = W20^T s0 + sc0   [scalars|gates, 512]
                    up0 = bkp.tile([128, 512], F32, tag="bank0",
                                   name=f"up0_{g}")
                    sRt = npl.tile([128, 512], BF16, tag="sRt")
                    nc.vector.tensor_add(out=sRt[:], in0=stsR[0][:],
                                         in1=stsR[1][:])
                    nc.vector.tensor_add(out=sRt[:], in0=sRt[:],
                                         in1=stsR[2][:])
                    nc.tensor.matmul(up0[:], lhsT=w20a_p[:],
                                     rhs=stsP[:], start=True, stop=False)
                    nc.tensor.matmul(up0[:], lhsT=w20b_p[:],
                                     rhs=sRt[:],
                                     start=False, stop=False)
                    nc.tensor.matmul(up0[:], lhsT=wsc0a[:],
                                     rhs=ytile[:, 0:512],
                                     start=False, stop=False)
                    nc.tensor.matmul(up0[:], lhsT=wsc0b[:],
                                     rhs=ytile[:, 512:1024],
                                     start=False, stop=True)

                    # up1: d0 rows 0:64, d1 rows 64:128 of up1a; d2 in up1b
                    up1a = bkp.tile([128, 512], F32, tag="bank1",
                                     name=f"up1a_{g}")
                    for d in (0, 1):
                        rows = slice(64 * d, 64 * d + 64)
                        if d == 0:
                            s1b_src, s1b_w = stsP[:], w21b_b[:]
                        else:
                            s1b_src, s1b_w = stsQ[:], w21b_t[:]
                        nc.tensor.matmul(up1a[rows, :], lhsT=w21a_t[:],
                                         rhs=stsR[d][:],
                                         start=True, stop=False)
                        nc.tensor.matmul(up1a[rows, :], lhsT=s1b_w, rhs=s1b_src,
                                         start=False, stop=False)
                        yo = 1024 * (1 + d)
                        nc.tensor.matmul(up1a[rows, :], lhsT=wsc1a[:],
                                         rhs=ytile[:, yo:yo + 512],
                                         start=False, stop=False)
                        nc.tensor.matmul(up1a[rows, :], lhsT=wsc1b[:],
                                         rhs=ytile[:, yo + 512:yo + 1024],
                                         start=False, stop=True)

                    # gate scalars/gates -> bf16
                    t0s = npl.tile([128, 512], BF16, tag="t0s")
                    nc.scalar.activation(t0s[:], up0[:], ACT_SILU)

                    up1b = wpp.tile([64, 512], F32, tag="wpb",
                                    name=f"up1b_{g}")
                    nc.tensor.matmul(up1b[:], lhsT=w21a_t[:],
                                     rhs=stsR[2][:], start=True, stop=False)
                    nc.tensor.matmul(up1b[:], lhsT=w21b_b[:],
                                     rhs=stsQ[:], start=False, stop=False)
                    nc.tensor.matmul(up1b[:], lhsT=wsc1a[:],
                                     rhs=ytile[:, 3072:3584],
                                     start=False, stop=False)
                    nc.tensor.matmul(up1b[:], lhsT=wsc1b[:],
                                     rhs=ytile[:, 3584:4096],
                                     start=False, stop=True)

                    # assemble: vectors = gates*t1 (t1 from psum); resnet add
                    outa = npl.tile([128, 512], BF16, tag="outa")
                    nc.vector.tensor_add(out=outa[0:64, :], in0=t0s[0:64, :],
                                         in1=xga[0:64, :])
                    nc.vector.tensor_tensor(out=outa[64:128, :],
                                            in0=t0s[64:128, :],
                                            in1=up1a[0:64, :], op=MULT)
                    nc.vector.tensor_add(out=outa[64:128, :], in0=outa[64:128, :],
                                         in1=xga[64:128, :])
                    outb = npl.tile([128, 512], BF16, tag="outb")
                    nc.vector.tensor_tensor(out=outb[0:64, :],
                                            in0=t0s[64:128, :],
                                            in1=up1a[64:128, :], op=MULT)
                    nc.vector.tensor_add(out=outb[0:64, :], in0=outb[0:64, :],
                                         in1=xgb[0:64, :])
                    nc.vector.tensor_tensor(out=outb[64:128, :],
                                            in0=t0s[64:128, :],
                                            in1=up1b[:], op=MULT)
                    nc.vector.tensor_add(out=outb[64:128, :], in0=outb[64:128, :],
                                         in1=xgb[64:128, :])

                nc.sync.dma_start(out=outT[0:128, cols], in_=outa[:])
                nc.sync.dma_start(out=outT[128:256, cols], in_=outb[:])

    nc.compile()
    return nc


# ---------------------------------------------------------------- host prep
def _pack_group(cols, T):
    """Greedy pack of sorted dst-cols into T tiles on the uniform grid.
    Returns per-tile edge index lists (positions into cols) or None."""
    starts = _grid_starts(T)
    res = []
    j, nE = 0, len(cols)
    for t in range(T):
        lo, hi = starts[t], starts[t] + SPAN
        tl = []
        while j < nE and len(tl) < 128 and cols[j] < hi:
            if cols[j] < lo:
                return None
            tl.append(j)
            j += 1
        res.append(tl)
    if j < nE:
        return None
    return res


def _host_prep(node_feats, node_attrs, edge_attrs, edge_embedding,
               W_lin1_0, W_lin1_1, W_mlp1, W_mlp2,
               W_lin2_0, W_lin2_1, W_sc0, W_sc1, edge_index):
    inv = 1.0 / np.sqrt(MUL)
    inv_e = 1.0 / np.sqrt(EDIM)
    inv2 = 1.0 / np.sqrt(2 * MUL)
    inv_n = 1.0 / np.sqrt(AVG_NEIGH)
    inv_sc = 1.0 / np.sqrt(MUL * NZ)

    # channel permutation: ours = [x0(64) | x1 d-major(192)]
    gidx = np.empty(256, np.int64)
    gidx[:64] = np.arange(64)
    for d in range(3):
        for u in range(64):
            gidx[64 + 64 * d + u] = 64 + 3 * u + d

    # permuted node feats in bf16 (row N = zero pad row for empty slots)
    xg_pad = np.zeros((N + 1, 256), np.float32)
    xg_pad[:N] = node_feats[:, gidx]
    xg_pad_b = xg_pad.astype(NP_BF16)

    xgf = np.zeros((NP_PAD, 256), np.float32)
    xgf[:N] = node_feats[:, gidx]
    xT = np.ascontiguousarray(xgf.T)
    xTf = xT.astype(NP_BF16)

    attT = np.zeros((NZ, NP_PAD), np.float32)
    attT[:, :N] = node_attrs.T.astype(np.float32)

    # ---- edge sorting and per-(core,group) packing
    src = edge_index[0].astype(np.int64)
    dst = edge_index[1].astype(np.int64)
    order = np.argsort(dst, kind="stable")
    src_s, dst_s = src[order], dst[order]
    ea_s = edge_attrs[order].astype(np.float32)
    emb_s = edge_embedding[order].astype(np.float32)

    bounds = np.searchsorted(dst_s, np.arange(0, NP_PAD + 1, GRP))
    all_cols = []
    T = 2
    for c in range(CORES):
        for g in range(NG):
            gi = c * NG + g
            s, e = bounds[gi], bounds[gi + 1]
            cols = (dst_s[s:e] - gi * GRP).astype(int)
            all_cols.append(cols)
            Tg = max(1, int(np.ceil(len(cols) / 128)))
            while Tg < 96 and _pack_group(cols, Tg) is None:
                Tg += 1
            T = max(T, Tg)
    T = T + (T % 2)  # even
    while any(_pack_group(cols, T) is None for cols in all_cols):
        T += 2

    C = T * 128

    per_core = []
    for c in range(CORES):
        slot_src = np.full((NG, C), N, np.int64)   # default -> zero row
        oh = np.zeros((NG, T, 128, 4 * SPAN), np.float32)
        embw = np.zeros((NG, EDIM, C), np.float32)
        starts = _grid_starts(T)
        for g in range(NG):
            gi = c * NG + g
            s = bounds[gi]
            cols = all_cols[gi]
            pk = _pack_group(cols, T)
            assert pk is not None
            for t, tl in enumerate(pk):
                if not tl:
                    continue
                idx = np.asarray(tl, np.int64)
                p = np.arange(len(tl))
                slot = t * 128 + p
                sn = src_s[s + idx]
                slot_src[g, slot] = sn
                embw[g, :, slot] = emb_s[s + idx]
                cc = cols[idx] - starts[t]
                oh[g, t, p, cc] = ea_s[s + idx, 0]               # oh*a0
                for d in range(3):
                    oh[g, t, p, SPAN * (1 + d) + cc] = ea_s[s + idx, 1 + d]
        # pre-gathered x per slot, channel-major [NG, 256, C]
        xe = xg_pad_b[slot_src.reshape(-1)]          # [NG*C, 256] bf16
        xe_dev = np.ascontiguousarray(
            xe.reshape(NG, C, 256).transpose(0, 2, 1))
        # device layout [NG, 128, T*4*SPAN]
        oh_dev = oh.transpose(0, 2, 1, 3).reshape(NG, 128, T * 4 * SPAN)
        # embd blockdiag3 layout: [NG, 32*(t%3)+j, 128*(t//3)+n] for
        # slot t*128 + n (rows 8:32 of each 32-block and 96:128 zero)
        NT3 = (T + 2) // 3
        embp = np.zeros((NG, EDIM, NT3 * 384), np.float32)
        embp[:, :, :C] = embw
        embp = embp.reshape(NG, EDIM, NT3, 3, 128)
        emb2 = np.zeros((NG, 4, 32, NT3, 128), np.float32)
        emb2[:, :3, :EDIM] = embp.transpose(0, 3, 1, 2, 4)
        emb2 = emb2.reshape(NG, 128, NT3 * 128)
        per_core.append(dict(
            xeT=xe_dev,
            ohd=np.ascontiguousarray(oh_dev).astype(NP_BF16),
            embd=np.ascontiguousarray(emb2).astype(NP_BF16),
        ))

    # ---- weights
    W10s = (W_lin1_0 * inv).astype(np.float32)
    W11s = (W_lin1_1 * inv).astype(np.float32)
    wblk1 = np.zeros((128, 128), np.float32)
    wblk1[:64, :64] = W10s
    wblk1[64:, 64:] = W11s
    wblk2 = np.zeros((128, 128), np.float32)
    wblk2[:64, :64] = W11s
    wblk2[64:, 64:] = W11s
    wm1s = (W_mlp1 * inv_e).astype(np.float32)
    wm1 = np.zeros((128, 128), np.float32)
    for q in range(3):
        for r in range(4):
            wm1[32 * q:32 * q + EDIM,
                32 * q + 8 * r:32 * q + 8 * r + EDIM] = wm1s
    wm1 = wm1.astype(NP_BF16)
    w1 = W_mlp2[:, 0:64]
    w2 = W_mlp2[:, 64:128]
    w3 = W_mlp2[:, 128:192]
    w4 = W_mlp2[:, 192:256]
    # wp cols: [w1 | w3 | w2 | w4'], replicated in 4 partition quadrants
    wbig8 = (np.concatenate([w1, w3, w2, w4 * INV_SQRT3], axis=1)
             * inv_e).astype(np.float32)
    wbig = np.zeros((128, 256), np.float32)
    for q in range(3):
        for r in range(4):
            wbig[32 * q + 8 * r:32 * q + 8 * r + EDIM] = wbig8 * 0.25
    wbig = wbig.astype(NP_BF16)
    w20s = (W_lin2_0 * inv2 * inv_n).astype(np.float32)
    w21s = (W_lin2_1 * inv2 * inv_n).astype(np.float32)
    z64x128 = np.zeros((64, 128), np.float32)
    z64x64 = np.zeros((64, 64), np.float32)
    w20a_p = np.concatenate([w20s[0:64], z64x128]).astype(NP_BF16)
    w20b_p = np.concatenate([z64x128, w20s[64:128]]).astype(NP_BF16)
    w21a_t = np.concatenate([w21s[0:64], z64x64]).astype(NP_BF16)
    w21b_t = np.concatenate([w21s[64:128], z64x64]).astype(NP_BF16)
    w21b_b = np.concatenate([z64x64, w21s[64:128]]).astype(NP_BF16)
    wsc0z = (np.transpose(W_sc0, (1, 0, 2)).reshape(NZ * MUL, 2 * MUL)
             * inv_sc).astype(NP_BF16)
    wsc1z = (np.transpose(W_sc1, (1, 0, 2)).reshape(NZ * MUL, MUL)
             * inv_sc).astype(NP_BF16)
    wsc0 = np.stack([wsc0z[:128], wsc0z[128:]])
    wsc1 = np.stack([wsc1z[:128], wsc1z[128:]])

    shared = dict(wblk1=wblk1.astype(NP_BF16),
                  wblk2=wblk2.astype(NP_BF16), wm1=wm1, wbig=wbig,
                  w20a_p=w20a_p, w20b_p=w20b_p, w21a_t=w21a_t,
                  w21b_t=w21b_t, w21b_b=w21b_b, wsc0=wsc0, wsc1=wsc1)
    in_maps = []
    for c in range(CORES):
        m = dict(shared)
        csl = slice(c * NPC, (c + 1) * NPC)
        m["xTb"] = np.ascontiguousarray(xTf[:, csl])
        # host-precomputed self-connection inputs y = x_block * z_attr
        # block (k, h): cols 512*(2k+h), rows z*64+u (z = 2h + p//64)
        ycore = np.empty((128, NPC, 8), np.float32)
        xTc = xT[:, csl]
        atc = attT[:, csl]
        for k in range(4):
            xk = xTc[64 * k:64 * k + 64]
            for h in range(2):
                ycore[0:64, :, 2 * k + h] = xk * atc[2 * h]
                ycore[64:128, :, 2 * k + h] = xk * atc[2 * h + 1]
        yd = ycore.reshape(128, NG, 512, 8).transpose(1, 0, 3, 2).reshape(
            NG, 128, 8 * 512)
        m["ydram"] = np.ascontiguousarray(yd).astype(NP_BF16)
        m.update(per_core[c])
        in_maps.append(m)
    return T, in_maps, gidx


_PROGRAM_CACHE = {}


def kernel(**inputs):
    global LAST_RESULT
    _install_profile_hook()

    args = {k: np.asarray(v) for k, v in inputs.items()}
    T, in_maps, gidx = _host_prep(
        args["node_feats"].astype(np.float32),
        args["node_attrs"].astype(np.float32),
        args["edge_attrs"].astype(np.float32),
        args["edge_embedding"].astype(np.float32),
        args["W_lin1_0"].astype(np.float32),
        args["W_lin1_1"].astype(np.float32),
        args["W_mlp1"].astype(np.float32),
        args["W_mlp2"].astype(np.float32),
        args["W_lin2_0"].astype(np.float32),
        args["W_lin2_1"].astype(np.float32),
        args["W_sc0"].astype(np.float32),
        args["W_sc1"].astype(np.float32),
        args["edge_index"])

    if T not in _PROGRAM_CACHE:
        _PROGRAM_CACHE[T] = _build_program(T)
    nc = _PROGRAM_CACHE[T]

    trace = bool(int(os.environ.get("BASS_TRACE", "0")))
    res = run_bass_kernel_spmd(nc, in_maps, core_ids=list(range(CORES)),
                               trace=trace)
    LAST_RESULT = res

    outT = np.concatenate(
        [res.results[c]["outT"].astype(np.float32) for c in range(CORES)],
        axis=1)                            # [256, NP_PAD]
    full = outT.T[:N]                      # [N, 256] in our channel order
    out = np.empty((N, 256), np.float32)
    out[:, gidx] = full
    return out
